# revision 7
# baseline (speedup 1.0000x reference)
"""GNN message-passing (NodeModel) Trainium2 kernel.

Strategy ("shard nodes, bucket edges" — a refinement of the edge-sharding hint
that removes the all-reduce entirely):
  * Host buckets edges by destination-node bucket of 128 nodes (a counting
    sort by `row >> 7`).  Buckets are distributed contiguously over the 8
    cores, so each core owns a contiguous 1/8 slice of the node space and
    ALL edges that point into it.  No cross-core reduction is needed.
  * On device, each 128-node bucket's segment-sum is computed on the tensor
    engine: for each 128-edge tile, a one-hot "placement" matrix
    P[e, n] = (local_id[e] == n) is built with a DVE is_equal against an
    iota row, and  aggT += attr_tile.T @ P  accumulates in PSUM.  Padding
    edges carry local id -1 and so contribute nothing.
  * The 2-layer MLP runs on the same core over its node slice, entirely in
    feature-major (transposed) layout:  hT = silu(W1x.T@xT + W1a.T@aggT + b1),
    outT = W2.T @ hT + b2.
  * Device outputs are feature-major [64, nodes/core]; the host transposes
    and assembles the full (out, combined) pair.
"""

import functools
import os
import sys

import numpy as np


def _ensure_path():
    try:
        import concourse  # noqa: F401
    except ImportError:
        for p in ("/opt/trn_rl_repo", "/root/.axon_site/_ro/trn_rl_repo"):
            if os.path.isdir(p):
                sys.path.insert(0, p)
                break


P = 128
D = 64
N_CORES = 8

# Stash of the last BassKernelResults (for test harness introspection).
LAST_RESULTS = None


@functools.lru_cache(maxsize=None)
def _build_program(T: int, NB: int):
    """Build the Bass program.

    T  = edge tiles (of 128 edges) per node bucket
    NB = node buckets (of 128 nodes) per core
    """
    _ensure_path()
    import concourse.tile as tile
    from concourse import bacc, mybir
    from contextlib import ExitStack

    f32 = mybir.dt.float32
    NN = NB * P  # nodes per core

    nc = bacc.Bacc("TRN2", target_bir_lowering=False, debug=False)

    attr_d = nc.declare_dram_parameter("attr", [NB * P, T * D], f32, isOutput=False)
    idx_d = nc.declare_dram_parameter("idx", [P, NB * T], f32, isOutput=False)
    xt_d = nc.declare_dram_parameter("xt", [D, NN], f32, isOutput=False)
    w1x_d = nc.declare_dram_parameter("w1x", [D, D], f32, isOutput=False)
    w1a_d = nc.declare_dram_parameter("w1a", [D, D], f32, isOutput=False)
    w2_d = nc.declare_dram_parameter("w2", [D, D], f32, isOutput=False)
    b1_d = nc.declare_dram_parameter("b1", [D, 1], f32, isOutput=False)
    b2_d = nc.declare_dram_parameter("b2", [D, 1], f32, isOutput=False)
    iota_d = nc.declare_dram_parameter("iota", [P, P], f32, isOutput=False)
    aggt_o = nc.declare_dram_parameter("aggt", [D, NN], f32, isOutput=True)
    outt_o = nc.declare_dram_parameter("outt", [D, NN], f32, isOutput=True)

    with tile.TileContext(nc) as tc, ExitStack() as ctx:
        consts = ctx.enter_context(tc.tile_pool(name="consts", bufs=1))
        attr_pool = ctx.enter_context(tc.tile_pool(name="attr", bufs=3))
        plc_pool = ctx.enter_context(tc.tile_pool(name="plc", bufs=2))
        sbout_pool = ctx.enter_context(tc.tile_pool(name="sbout", bufs=3))
        ps_agg = ctx.enter_context(tc.tile_pool(name="ps_agg", bufs=2, space="PSUM"))
        ps_mlp = ctx.enter_context(tc.tile_pool(name="ps_mlp", bufs=2, space="PSUM"))

        iota_sb = consts.tile([P, P], f32)
        nc.sync.dma_start(iota_sb[:], iota_d[:, :])
        b1_sb = consts.tile([D, 1], f32)
        nc.sync.dma_start(b1_sb[:], b1_d[:, :])
        b2_sb = consts.tile([D, 1], f32)
        nc.sync.dma_start(b2_sb[:], b2_d[:, :])
        idx_sb = consts.tile([P, NB * T], f32)
        nc.sync.dma_start(idx_sb[:], idx_d[:, :])
        w1x_sb = consts.tile([D, D], f32)
        nc.sync.dma_start(w1x_sb[:], w1x_d[:, :])
        w1a_sb = consts.tile([D, D], f32)
        nc.sync.dma_start(w1a_sb[:], w1a_d[:, :])
        w2_sb = consts.tile([D, D], f32)
        nc.sync.dma_start(w2_sb[:], w2_d[:, :])
        xt_sb = consts.tile([D, NN], f32)
        nc.sync.dma_start(xt_sb[:], xt_d[:, :])

        for b in range(NB):
            attr_sb = attr_pool.tile([P, T * D], f32)
            nc.sync.dma_start(attr_sb[:], attr_d[b * P : (b + 1) * P, :])

            # one batched DVE op builds all T placement one-hots of the bucket
            plc = plc_pool.tile([P, T, P], f32)
            nc.vector.tensor_tensor(
                out=plc[:],
                in0=idx_sb[:, b * T : (b + 1) * T].unsqueeze(2).to_broadcast([P, T, P]),
                in1=iota_sb[:].unsqueeze(1).to_broadcast([P, T, P]),
                op=mybir.AluOpType.is_equal,
            )

            agg_ps = ps_agg.tile([D, P], f32)
            for t in range(T):
                nc.tensor.matmul(
                    out=agg_ps[:],
                    lhsT=attr_sb[:, t * D : (t + 1) * D],
                    rhs=plc[:, t, :],
                    start=(t == 0),
                    stop=(t == T - 1),
                )

            agg_sb = sbout_pool.tile([D, P], f32, tag="agg")
            nc.vector.tensor_copy(agg_sb[:], agg_ps[:])
            nc.sync.dma_start(aggt_o[:, b * P : (b + 1) * P], agg_sb[:])

            h_ps = ps_mlp.tile([D, P], f32, tag="h")
            nc.tensor.matmul(
                out=h_ps[:], lhsT=w1x_sb[:], rhs=xt_sb[:, b * P : (b + 1) * P],
                start=True, stop=False,
            )
            nc.tensor.matmul(
                out=h_ps[:], lhsT=w1a_sb[:], rhs=agg_sb[:],
                start=False, stop=True,
            )
            h_sb = sbout_pool.tile([D, P], f32, tag="hsb")
            nc.scalar.activation(
                out=h_sb[:], in_=h_ps[:],
                func=mybir.ActivationFunctionType.Silu,
                bias=b1_sb[:],
            )
            o_ps = ps_mlp.tile([D, P], f32, tag="o")
            nc.tensor.matmul(out=o_ps[:], lhsT=w2_sb[:], rhs=h_sb[:], start=True, stop=True)
            o_sb = sbout_pool.tile([D, P], f32, tag="osb")
            nc.scalar.activation(
                out=o_sb[:], in_=o_ps[:],
                func=mybir.ActivationFunctionType.Identity,
                bias=b2_sb[:],
            )
            nc.sync.dma_start(outt_o[:, b * P : (b + 1) * P], o_sb[:])

    nc.compile()
    return nc


def _prepare(edge_index, edge_attr, x):
    """Host-side bucketing/sharding. Returns (in_maps_partial, meta)."""
    n_nodes = x.shape[0]
    n_edges = edge_index.shape[1]

    nb_total = -(-n_nodes // P)  # buckets of 128 nodes
    nb_total = -(-nb_total // N_CORES) * N_CORES  # round to multiple of n_cores
    NB = nb_total // N_CORES  # buckets per core
    n_pad = nb_total * P

    row = np.asarray(edge_index[0], dtype=np.int64)
    bucket = row >> 7
    order = np.argsort(bucket, kind="stable")
    counts = np.bincount(bucket, minlength=nb_total)
    T = max(1, int(-(-counts.max() // P)))
    S = T * P

    starts = np.zeros(nb_total, np.int64)
    starts[1:] = np.cumsum(counts)[:-1]
    bs = bucket[order]
    dest = bs * S + (np.arange(n_edges, dtype=np.int64) - starts[bs])
    perm = np.full(nb_total * S, -1, np.int64)
    perm[dest] = order
    valid = perm >= 0
    perm_c = np.where(valid, perm, 0)

    attr_pad = np.asarray(edge_attr, dtype=np.float32)[perm_c]
    attr_dram = attr_pad.reshape(nb_total * P, T * D)

    idxl = np.where(valid, (row[perm_c] & 127).astype(np.float32), np.float32(-1.0))
    # per-bucket [128, T] layout, then per-core [128, NB*T]
    idx_bpt = idxl.reshape(nb_total, P, T)

    x_pad = np.zeros((n_pad, D), np.float32)
    x_pad[:n_nodes] = np.asarray(x, dtype=np.float32)

    nn_core = NB * P
    per_core = []
    for k in range(N_CORES):
        r0, r1 = k * NB * P, (k + 1) * NB * P
        xt_k = np.ascontiguousarray(x_pad[k * nn_core : (k + 1) * nn_core].T)
        idx_k = np.ascontiguousarray(
            idx_bpt[k * NB : (k + 1) * NB].transpose(1, 0, 2).reshape(P, NB * T)
        )
        per_core.append({
            "attr": attr_dram[r0:r1],
            "idx": idx_k,
            "xt": xt_k,
        })
    meta = dict(T=T, NB=NB, n_pad=n_pad, nn_core=nn_core)
    return per_core, meta


def kernel(edge_index, edge_attr, x, W1, b1, W2, b2):
    global LAST_RESULTS
    _ensure_path()
    from concourse.bass_utils import run_bass_kernel_spmd

    edge_index = np.asarray(edge_index)
    edge_attr = np.asarray(edge_attr, dtype=np.float32)
    x = np.asarray(x, dtype=np.float32)
    W1 = np.asarray(W1, dtype=np.float32)
    b1 = np.asarray(b1, dtype=np.float32)
    W2 = np.asarray(W2, dtype=np.float32)
    b2 = np.asarray(b2, dtype=np.float32)

    n_nodes = x.shape[0]
    per_core, meta = _prepare(edge_index, edge_attr, x)
    T, NB, nn_core = meta["T"], meta["NB"], meta["nn_core"]

    iota = np.ascontiguousarray(
        np.broadcast_to(np.arange(P, dtype=np.float32), (P, P))
    )
    shared = {
        "w1x": np.ascontiguousarray(W1[:D]),
        "w1a": np.ascontiguousarray(W1[D:]),
        "w2": np.ascontiguousarray(W2),
        "b1": np.ascontiguousarray(b1.reshape(D, 1)),
        "b2": np.ascontiguousarray(b2.reshape(D, 1)),
        "iota": iota,
    }
    in_maps = [{**pc, **shared} for pc in per_core]

    nc = _build_program(T, NB)
    trace = bool(int(os.environ.get("KBENCH_TRACE", "0")))
    LAST_RESULTS = run_bass_kernel_spmd(
        nc, in_maps, list(range(N_CORES)), trace=trace
    )
    results = LAST_RESULTS.results

    agg = np.empty((meta["n_pad"], D), np.float32)
    out = np.empty((meta["n_pad"], D), np.float32)
    for k in range(N_CORES):
        agg[k * nn_core : (k + 1) * nn_core] = results[k]["aggt"].T
        out[k * nn_core : (k + 1) * nn_core] = results[k]["outt"].T

    combined = np.concatenate([x, agg[:n_nodes]], axis=1)
    return out[:n_nodes], combined


# revision 9
# speedup vs baseline: 1.5460x; 1.5460x over previous
"""GNN message-passing (NodeModel) Trainium2 kernel.

Strategy ("shard nodes, bucket edges" — a refinement of the edge-sharding hint
that removes the all-reduce entirely):
  * Host buckets edges by destination-node bucket of 128 nodes (a counting
    sort by `row >> 7`).  Buckets are distributed contiguously over the 8
    cores, so each core owns a contiguous 1/8 slice of the node space and
    ALL edges that point into it.  No cross-core reduction is needed.
  * On device, each 128-node bucket's segment-sum runs on the tensor engine:
    for each 128-edge tile, a one-hot "placement" matrix
    P[e, n] = (local_id[e] == n) is built with a DVE is_equal against an
    iota row, and  aggT += attr_tile.T @ P  accumulates in PSUM.  Padding
    edges carry local id -1 and so contribute nothing.
  * Precision: edge attrs are split on the host into bf16 hi + lo parts
    (attr ≈ hi + lo to ~2^-16 relative).  The stationary operand packs
    [hi | lo] as 128 columns, so ONE bf16 matmul per edge tile produces
    both partial aggregates (PSUM rows 0-63 = hi, 64-127 = lo); a DVE add
    folds them to fp32.  This halves tensor-engine time vs fp32 matmuls
    (which lower to 2 HW passes) while keeping ~1e-5 relative accuracy.
  * The 2-layer MLP runs on the same core over its node slice, entirely in
    feature-major (transposed) layout:  hT = silu(W1x.T@xT + W1a.T@aggT + b1),
    outT = W2.T @ hT + b2  (fp32).
  * Device outputs are feature-major [64, nodes/core]; the host transposes
    and assembles the full (out, combined) pair.
"""

import functools
import os
import sys

import numpy as np


def _ensure_path():
    try:
        import concourse  # noqa: F401
    except ImportError:
        for p in ("/opt/trn_rl_repo", "/root/.axon_site/_ro/trn_rl_repo"):
            if os.path.isdir(p):
                sys.path.insert(0, p)
                break


P = 128
D = 64
N_CORES = 8

# Stash of the last BassKernelResults (for test harness introspection).
LAST_RESULTS = None


def _bf16_split(a):
    """Round-to-nearest-even split of fp32 `a` into bf16 hi/lo bit patterns."""
    u = a.view(np.uint32)
    hi_u = ((u + (((u >> 16) & 1) + 0x7FFF)) >> 16).astype(np.uint16)
    hi_f = (hi_u.astype(np.uint32) << 16).view(np.float32)
    r = np.asarray(a - hi_f, dtype=np.float32)
    ur = r.view(np.uint32)
    lo_u = ((ur + (((ur >> 16) & 1) + 0x7FFF)) >> 16).astype(np.uint16)
    return hi_u, lo_u


@functools.lru_cache(maxsize=None)
def _build_program(T: int, NB: int):
    """Build the Bass program.

    T  = edge tiles (of 128 edges) per node bucket
    NB = node buckets (of 128 nodes) per core
    """
    _ensure_path()
    import concourse.tile as tile
    from concourse import bacc, mybir
    from contextlib import ExitStack

    f32 = mybir.dt.float32
    bf16 = mybir.dt.bfloat16
    NN = NB * P  # nodes per core
    TC = T * P  # columns per bucket row in the hi|lo attr layout

    nc = bacc.Bacc("TRN2", target_bir_lowering=False, debug=False)

    attr_d = nc.declare_dram_parameter("attr", [NB * P, TC], bf16, isOutput=False)
    idx_d = nc.declare_dram_parameter("idx", [P, NB * T], bf16, isOutput=False)
    xt_d = nc.declare_dram_parameter("xt", [D, NN], f32, isOutput=False)
    w1x_d = nc.declare_dram_parameter("w1x", [D, D], f32, isOutput=False)
    w1a_d = nc.declare_dram_parameter("w1a", [D, D], f32, isOutput=False)
    w2_d = nc.declare_dram_parameter("w2", [D, D], f32, isOutput=False)
    b1_d = nc.declare_dram_parameter("b1", [D, 1], f32, isOutput=False)
    b2_d = nc.declare_dram_parameter("b2", [D, 1], f32, isOutput=False)
    iota_d = nc.declare_dram_parameter("iota", [P, P], bf16, isOutput=False)
    aggt_o = nc.declare_dram_parameter("aggt", [D, NN], f32, isOutput=True)
    outt_o = nc.declare_dram_parameter("outt", [D, NN], f32, isOutput=True)

    with tile.TileContext(nc) as tc, ExitStack() as ctx:
        consts = ctx.enter_context(tc.tile_pool(name="consts", bufs=1))
        attr_pool = ctx.enter_context(tc.tile_pool(name="attr", bufs=3))
        plc_pool = ctx.enter_context(tc.tile_pool(name="plc", bufs=2))
        sbout_pool = ctx.enter_context(tc.tile_pool(name="sbout", bufs=3))
        ps_agg = ctx.enter_context(tc.tile_pool(name="ps_agg", bufs=2, space="PSUM"))
        ps_mlp = ctx.enter_context(tc.tile_pool(name="ps_mlp", bufs=2, space="PSUM"))

        iota_sb = consts.tile([P, P], bf16)
        nc.sync.dma_start(iota_sb[:], iota_d[:, :])
        b1_sb = consts.tile([D, 1], f32)
        nc.sync.dma_start(b1_sb[:], b1_d[:, :])
        b2_sb = consts.tile([D, 1], f32)
        nc.sync.dma_start(b2_sb[:], b2_d[:, :])
        idx_sb = consts.tile([P, NB * T], bf16)
        nc.sync.dma_start(idx_sb[:], idx_d[:, :])
        w1x_sb = consts.tile([D, D], f32)
        nc.sync.dma_start(w1x_sb[:], w1x_d[:, :])
        w1a_sb = consts.tile([D, D], f32)
        nc.sync.dma_start(w1a_sb[:], w1a_d[:, :])
        w2_sb = consts.tile([D, D], f32)
        nc.sync.dma_start(w2_sb[:], w2_d[:, :])
        xt_sb = consts.tile([D, NN], f32)
        nc.sync.dma_start(xt_sb[:], xt_d[:, :])

        for b in range(NB):
            attr_sb = attr_pool.tile([P, TC], bf16)
            nc.sync.dma_start(attr_sb[:], attr_d[b * P : (b + 1) * P, :])

            # one batched DVE op builds all T placement one-hots of the bucket
            plc = plc_pool.tile([P, T, P], bf16)
            nc.vector.tensor_tensor(
                out=plc[:],
                in0=idx_sb[:, b * T : (b + 1) * T].unsqueeze(2).to_broadcast([P, T, P]),
                in1=iota_sb[:].unsqueeze(1).to_broadcast([P, T, P]),
                op=mybir.AluOpType.is_equal,
            )

            agg_ps = ps_agg.tile([P, P], f32)
            for t in range(T):
                nc.tensor.matmul(
                    out=agg_ps[:],
                    lhsT=attr_sb[:, t * P : (t + 1) * P],
                    rhs=plc[:, t, :],
                    start=(t == 0),
                    stop=(t == T - 1),
                )

            # fold hi (rows 0-63) + lo (rows 64-127) partial sums to fp32
            # (only one operand may be in PSUM: stage hi through ACT first)
            agg_hi = sbout_pool.tile([D, P], f32, tag="agghi")
            nc.scalar.activation(
                out=agg_hi[:], in_=agg_ps[0:D, :],
                func=mybir.ActivationFunctionType.Copy,
            )
            agg_sb = sbout_pool.tile([D, P], f32, tag="agg")
            nc.vector.tensor_add(agg_sb[:], agg_hi[:], agg_ps[D : 2 * D, :])
            nc.sync.dma_start(aggt_o[:, b * P : (b + 1) * P], agg_sb[:])

            h_ps = ps_mlp.tile([D, P], f32, tag="h")
            nc.tensor.matmul(
                out=h_ps[:], lhsT=w1x_sb[:], rhs=xt_sb[:, b * P : (b + 1) * P],
                start=True, stop=False,
            )
            nc.tensor.matmul(
                out=h_ps[:], lhsT=w1a_sb[:], rhs=agg_sb[:],
                start=False, stop=True,
            )
            h_sb = sbout_pool.tile([D, P], f32, tag="hsb")
            nc.scalar.activation(
                out=h_sb[:], in_=h_ps[:],
                func=mybir.ActivationFunctionType.Silu,
                bias=b1_sb[:],
            )
            o_ps = ps_mlp.tile([D, P], f32, tag="o")
            nc.tensor.matmul(out=o_ps[:], lhsT=w2_sb[:], rhs=h_sb[:], start=True, stop=True)
            o_sb = sbout_pool.tile([D, P], f32, tag="osb")
            nc.scalar.activation(
                out=o_sb[:], in_=o_ps[:],
                func=mybir.ActivationFunctionType.Identity,
                bias=b2_sb[:],
            )
            nc.sync.dma_start(outt_o[:, b * P : (b + 1) * P], o_sb[:])

    nc.compile()
    return nc


def _prepare(edge_index, edge_attr, x):
    """Host-side bucketing/sharding. Returns (in_maps_partial, meta)."""
    import ml_dtypes

    n_nodes = x.shape[0]
    n_edges = edge_index.shape[1]

    nb_total = -(-n_nodes // P)  # buckets of 128 nodes
    nb_total = -(-nb_total // N_CORES) * N_CORES  # round to multiple of n_cores
    NB = nb_total // N_CORES  # buckets per core
    n_pad = nb_total * P

    row = np.asarray(edge_index[0], dtype=np.int64)
    bucket = row >> 7
    order = np.argsort(bucket, kind="stable")
    counts = np.bincount(bucket, minlength=nb_total)
    T = max(1, int(-(-counts.max() // P)))
    S = T * P

    starts = np.zeros(nb_total, np.int64)
    starts[1:] = np.cumsum(counts)[:-1]
    bs = bucket[order]
    dest = bs * S + (np.arange(n_edges, dtype=np.int64) - starts[bs])
    perm = np.full(nb_total * S, -1, np.int64)
    perm[dest] = order
    valid = perm >= 0
    perm_c = np.where(valid, perm, 0)

    attr_pad = np.ascontiguousarray(np.asarray(edge_attr, dtype=np.float32)[perm_c])
    hi_u, lo_u = _bf16_split(attr_pad)
    attr_hl = np.empty((nb_total * S, 2 * D), np.uint16)
    attr_hl[:, :D] = hi_u
    attr_hl[:, D:] = lo_u
    attr_dram = attr_hl.reshape(nb_total * P, T * P).view(ml_dtypes.bfloat16)

    idxl = np.where(valid, (row[perm_c] & 127).astype(np.float32), np.float32(-1.0))
    idxl = idxl.astype(ml_dtypes.bfloat16)  # ids 0..127 and -1 are exact in bf16
    idx_bpt = idxl.reshape(nb_total, P, T)

    x_pad = np.zeros((n_pad, D), np.float32)
    x_pad[:n_nodes] = np.asarray(x, dtype=np.float32)

    nn_core = NB * P
    per_core = []
    for k in range(N_CORES):
        r0, r1 = k * NB * P, (k + 1) * NB * P
        xt_k = np.ascontiguousarray(x_pad[k * nn_core : (k + 1) * nn_core].T)
        idx_k = np.ascontiguousarray(
            idx_bpt[k * NB : (k + 1) * NB].transpose(1, 0, 2).reshape(P, NB * T)
        )
        per_core.append({
            "attr": attr_dram[r0:r1],
            "idx": idx_k,
            "xt": xt_k,
        })
    meta = dict(T=T, NB=NB, n_pad=n_pad, nn_core=nn_core)
    return per_core, meta


def kernel(edge_index, edge_attr, x, W1, b1, W2, b2):
    global LAST_RESULTS
    _ensure_path()
    import ml_dtypes
    from concourse.bass_utils import run_bass_kernel_spmd

    edge_index = np.asarray(edge_index)
    edge_attr = np.asarray(edge_attr, dtype=np.float32)
    x = np.asarray(x, dtype=np.float32)
    W1 = np.asarray(W1, dtype=np.float32)
    b1 = np.asarray(b1, dtype=np.float32)
    W2 = np.asarray(W2, dtype=np.float32)
    b2 = np.asarray(b2, dtype=np.float32)

    n_nodes = x.shape[0]
    per_core, meta = _prepare(edge_index, edge_attr, x)
    T, NB, nn_core = meta["T"], meta["NB"], meta["nn_core"]

    iota = np.ascontiguousarray(
        np.broadcast_to(
            np.arange(P, dtype=np.float32).astype(ml_dtypes.bfloat16), (P, P)
        )
    )
    shared = {
        "w1x": np.ascontiguousarray(W1[:D]),
        "w1a": np.ascontiguousarray(W1[D:]),
        "w2": np.ascontiguousarray(W2),
        "b1": np.ascontiguousarray(b1.reshape(D, 1)),
        "b2": np.ascontiguousarray(b2.reshape(D, 1)),
        "iota": iota,
    }
    in_maps = [{**pc, **shared} for pc in per_core]

    nc = _build_program(T, NB)
    trace = bool(int(os.environ.get("KBENCH_TRACE", "0")))
    LAST_RESULTS = run_bass_kernel_spmd(
        nc, in_maps, list(range(N_CORES)), trace=trace
    )
    results = LAST_RESULTS.results

    agg = np.empty((meta["n_pad"], D), np.float32)
    out = np.empty((meta["n_pad"], D), np.float32)
    for k in range(N_CORES):
        agg[k * nn_core : (k + 1) * nn_core] = results[k]["aggt"].T
        out[k * nn_core : (k + 1) * nn_core] = results[k]["outt"].T

    combined = np.concatenate([x, agg[:n_nodes]], axis=1)
    return out[:n_nodes], combined


# revision 15
# speedup vs baseline: 1.7898x; 1.1577x over previous
"""GNN message-passing (NodeModel) Trainium2 kernel.

Strategy ("shard nodes, bucket edges" — a refinement of the edge-sharding hint
that removes the all-reduce entirely):
  * Host buckets edges by destination-node bucket of BW=64 nodes (a counting
    sort by `row >> 6`).  Buckets are distributed contiguously over the 8
    cores, so each core owns a contiguous 1/8 slice of the node space and
    ALL edges that point into it.  No cross-core reduction is needed.
  * On device, each bucket's segment-sum runs on the tensor engine: for each
    128-edge tile, a one-hot "placement" matrix P[e, n] = (local_id[e] == n)
    is built with a DVE is_equal against an iota row, and
    aggT += attr_tile.T @ P accumulates in PSUM.  Padding edges carry local
    id -1 and contribute nothing.  BW=64 keeps the placement build (the DVE
    cost) at half of what 128-wide buckets would need.
  * Precision: edge attrs are split on the host into bf16 hi + lo parts
    (attr ≈ hi + lo to ~2^-16 relative).  The stationary operand packs
    [hi | lo] as 128 columns, so ONE bf16 matmul per edge tile produces
    both partial aggregates (PSUM rows 0-63 = hi, 64-127 = lo); an ACT copy
    plus DVE add folds them to fp32.  This halves tensor-engine time vs
    fp32 matmuls (which lower to 2 HW passes) at ~1e-5 relative accuracy.
  * The 2-layer MLP runs on the same core over its node slice, entirely in
    feature-major (transposed) layout:  hT = silu(W1x.T@xT + W1a.T@aggT + b1),
    outT = W2.T @ hT + b2  (fp32), batched 512 nodes per matmul.
  * Device outputs are feature-major [64, nodes/core]; the host transposes
    and assembles the full (out, combined) pair.
"""

import functools
import os
import sys

import numpy as np


def _ensure_path():
    try:
        import concourse  # noqa: F401
    except ImportError:
        for p in ("/opt/trn_rl_repo", "/root/.axon_site/_ro/trn_rl_repo"):
            if os.path.isdir(p):
                sys.path.insert(0, p)
                break


P = 128  # edge-tile size (contraction dim)
D = 64  # feature dim
BW = 64  # nodes per bucket (one-hot width)
N_CORES = 8

# Stash of the last BassKernelResults (for test harness introspection).
LAST_RESULTS = None


def _bf16_split(a):
    """Round-to-nearest-even split of fp32 `a` into bf16 hi/lo bit patterns."""
    u = a.view(np.uint32)
    hi_u = ((u + (((u >> 16) & 1) + 0x7FFF)) >> 16).astype(np.uint16)
    hi_f = (hi_u.astype(np.uint32) << 16).view(np.float32)
    r = np.asarray(a - hi_f, dtype=np.float32)
    ur = r.view(np.uint32)
    lo_u = ((ur + (((ur >> 16) & 1) + 0x7FFF)) >> 16).astype(np.uint16)
    return hi_u, lo_u


@functools.lru_cache(maxsize=None)
def _build_program(T: int, NB: int):
    """Build the Bass program.

    T  = edge tiles (of 128 edges) per node bucket
    NB = node buckets (of BW nodes) per core
    """
    _ensure_path()
    import concourse.tile as tile
    from concourse import bacc, mybir
    from contextlib import ExitStack

    f32 = mybir.dt.float32
    bf16 = mybir.dt.bfloat16
    NN = NB * BW  # nodes per core
    TC = T * P  # columns per bucket row in the hi|lo attr layout

    nc = bacc.Bacc("TRN2", target_bir_lowering=False, debug=False)

    attr_d = nc.declare_dram_parameter("attr", [NB * P, TC], bf16, isOutput=False)
    idx_d = nc.declare_dram_parameter("idx", [P, NB * T], bf16, isOutput=False)
    xt_d = nc.declare_dram_parameter("xt", [D, NN], f32, isOutput=False)
    w1x_d = nc.declare_dram_parameter("w1x", [D, D], f32, isOutput=False)
    w1a_d = nc.declare_dram_parameter("w1a", [D, D], f32, isOutput=False)
    w2_d = nc.declare_dram_parameter("w2", [D, D], f32, isOutput=False)
    b1_d = nc.declare_dram_parameter("b1", [D, 1], f32, isOutput=False)
    b2_d = nc.declare_dram_parameter("b2", [D, 1], f32, isOutput=False)
    iota_d = nc.declare_dram_parameter("iota", [P, BW], bf16, isOutput=False)
    aggt_o = nc.declare_dram_parameter("aggt", [D, NN], f32, isOutput=True)
    outt_o = nc.declare_dram_parameter("outt", [D, NN], f32, isOutput=True)

    with tile.TileContext(nc) as tc, ExitStack() as ctx:
        consts = ctx.enter_context(tc.tile_pool(name="consts", bufs=1))
        attr_pool = ctx.enter_context(tc.tile_pool(name="attr", bufs=6))
        plc_pool = ctx.enter_context(tc.tile_pool(name="plc", bufs=4))
        sbout_pool = ctx.enter_context(tc.tile_pool(name="sbout", bufs=3))
        ps_agg = ctx.enter_context(tc.tile_pool(name="ps_agg", bufs=2, space="PSUM"))
        ps_mlp = ctx.enter_context(tc.tile_pool(name="ps_mlp", bufs=2, space="PSUM"))

        iota_sb = consts.tile([P, BW], bf16)
        nc.sync.dma_start(iota_sb[:], iota_d[:, :])
        b1_sb = consts.tile([D, 1], f32)
        nc.sync.dma_start(b1_sb[:], b1_d[:, :])
        b2_sb = consts.tile([D, 1], f32)
        nc.sync.dma_start(b2_sb[:], b2_d[:, :])
        idx_sb = consts.tile([P, NB * T], bf16)
        nc.sync.dma_start(idx_sb[:], idx_d[:, :])
        w1x_sb = consts.tile([D, D], f32)
        nc.sync.dma_start(w1x_sb[:], w1x_d[:, :])
        w1a_sb = consts.tile([D, D], f32)
        nc.sync.dma_start(w1a_sb[:], w1a_d[:, :])
        w2_sb = consts.tile([D, D], f32)
        nc.sync.dma_start(w2_sb[:], w2_d[:, :])
        xt_sb = consts.tile([D, NN], f32)
        nc.sync.dma_start(xt_sb[:], xt_d[:, :])

        # MLP is batched over groups of buckets (up to 512 nodes per matmul)
        G = 512 // BW
        groups = [list(range(g0, min(g0 + G, NB))) for g0 in range(0, NB, G)]
        for grp in groups:
            W = len(grp) * BW
            agg4 = sbout_pool.tile([D, G * BW], f32, tag="agg")
            for q, b in enumerate(grp):
                attr_sb = attr_pool.tile([P, TC], bf16)
                nc.sync.dma_start(attr_sb[:], attr_d[b * P : (b + 1) * P, :])

                # one batched DVE op builds all T placement one-hots
                plc = plc_pool.tile([P, T, BW], bf16)
                nc.vector.tensor_tensor(
                    out=plc[:],
                    in0=idx_sb[:, b * T : (b + 1) * T]
                    .unsqueeze(2)
                    .to_broadcast([P, T, BW]),
                    in1=iota_sb[:].unsqueeze(1).to_broadcast([P, T, BW]),
                    op=mybir.AluOpType.is_equal,
                )

                agg_ps = ps_agg.tile([P, BW], f32)
                for t in range(T):
                    nc.tensor.matmul(
                        out=agg_ps[:],
                        lhsT=attr_sb[:, t * P : (t + 1) * P],
                        rhs=plc[:, t, :],
                        start=(t == 0),
                        stop=(t == T - 1),
                    )

                # fold hi (rows 0-63) + lo (rows 64-127) partial sums to fp32
                # (only one operand may be in PSUM: stage hi through ACT first)
                agg_hi = sbout_pool.tile([D, BW], f32, tag="agghi")
                nc.scalar.activation(
                    out=agg_hi[:], in_=agg_ps[0:D, :],
                    func=mybir.ActivationFunctionType.Copy,
                )
                nc.vector.tensor_add(
                    agg4[:, q * BW : (q + 1) * BW], agg_hi[:], agg_ps[D : 2 * D, :]
                )

            c0 = grp[0] * BW
            nc.sync.dma_start(aggt_o[:, c0 : c0 + W], agg4[:, :W])

            h_ps = ps_mlp.tile([D, G * BW], f32, tag="h")
            nc.tensor.matmul(
                out=h_ps[:, :W], lhsT=w1x_sb[:], rhs=xt_sb[:, c0 : c0 + W],
                start=True, stop=False,
            )
            nc.tensor.matmul(
                out=h_ps[:, :W], lhsT=w1a_sb[:], rhs=agg4[:, :W],
                start=False, stop=True,
            )
            h_sb = sbout_pool.tile([D, G * BW], f32, tag="hsb")
            nc.scalar.activation(
                out=h_sb[:, :W], in_=h_ps[:, :W],
                func=mybir.ActivationFunctionType.Silu,
                bias=b1_sb[:],
            )
            o_ps = ps_mlp.tile([D, G * BW], f32, tag="o")
            nc.tensor.matmul(
                out=o_ps[:, :W], lhsT=w2_sb[:], rhs=h_sb[:, :W],
                start=True, stop=True,
            )
            o_sb = sbout_pool.tile([D, G * BW], f32, tag="osb")
            nc.scalar.activation(
                out=o_sb[:, :W], in_=o_ps[:, :W],
                func=mybir.ActivationFunctionType.Identity,
                bias=b2_sb[:],
            )
            nc.sync.dma_start(outt_o[:, c0 : c0 + W], o_sb[:, :W])

    nc.compile()
    return nc


def _prepare(edge_index, edge_attr, x):
    """Host-side bucketing/sharding. Returns (in_maps_partial, meta)."""
    import ml_dtypes

    n_nodes = x.shape[0]
    n_edges = edge_index.shape[1]

    shift = BW.bit_length() - 1
    nb_total = -(-n_nodes // BW)  # buckets of BW nodes
    nb_total = -(-nb_total // N_CORES) * N_CORES  # round to multiple of n_cores
    NB = nb_total // N_CORES  # buckets per core
    n_pad = nb_total * BW

    row = np.asarray(edge_index[0], dtype=np.int64)
    bucket = row >> shift
    order = np.argsort(bucket, kind="stable")
    counts = np.bincount(bucket, minlength=nb_total)
    T = max(1, int(-(-counts.max() // P)))
    S = T * P

    starts = np.zeros(nb_total, np.int64)
    starts[1:] = np.cumsum(counts)[:-1]
    bs = bucket[order]
    dest = bs * S + (np.arange(n_edges, dtype=np.int64) - starts[bs])
    perm = np.full(nb_total * S, -1, np.int64)
    perm[dest] = order
    valid = perm >= 0
    perm_c = np.where(valid, perm, 0)

    attr_pad = np.ascontiguousarray(np.asarray(edge_attr, dtype=np.float32)[perm_c])
    hi_u, lo_u = _bf16_split(attr_pad)
    attr_hl = np.empty((nb_total * S, 2 * D), np.uint16)
    attr_hl[:, :D] = hi_u
    attr_hl[:, D:] = lo_u
    attr_dram = attr_hl.reshape(nb_total * P, T * P).view(ml_dtypes.bfloat16)

    idxl = np.where(
        valid, (row[perm_c] & (BW - 1)).astype(np.float32), np.float32(-1.0)
    )
    idxl = idxl.astype(ml_dtypes.bfloat16)  # ids 0..BW-1 and -1 are exact in bf16
    idx_bpt = idxl.reshape(nb_total, P, T)

    x_pad = np.zeros((n_pad, D), np.float32)
    x_pad[:n_nodes] = np.asarray(x, dtype=np.float32)

    nn_core = NB * BW
    per_core = []
    for k in range(N_CORES):
        r0, r1 = k * NB * P, (k + 1) * NB * P
        xt_k = np.ascontiguousarray(x_pad[k * nn_core : (k + 1) * nn_core].T)
        idx_k = np.ascontiguousarray(
            idx_bpt[k * NB : (k + 1) * NB].transpose(1, 0, 2).reshape(P, NB * T)
        )
        per_core.append({
            "attr": attr_dram[r0:r1],
            "idx": idx_k,
            "xt": xt_k,
        })
    meta = dict(T=T, NB=NB, n_pad=n_pad, nn_core=nn_core)
    return per_core, meta


def kernel(edge_index, edge_attr, x, W1, b1, W2, b2):
    global LAST_RESULTS
    _ensure_path()
    import ml_dtypes
    from concourse.bass_utils import run_bass_kernel_spmd

    edge_index = np.asarray(edge_index)
    edge_attr = np.asarray(edge_attr, dtype=np.float32)
    x = np.asarray(x, dtype=np.float32)
    W1 = np.asarray(W1, dtype=np.float32)
    b1 = np.asarray(b1, dtype=np.float32)
    W2 = np.asarray(W2, dtype=np.float32)
    b2 = np.asarray(b2, dtype=np.float32)

    n_nodes = x.shape[0]
    per_core, meta = _prepare(edge_index, edge_attr, x)
    T, NB, nn_core = meta["T"], meta["NB"], meta["nn_core"]

    iota = np.ascontiguousarray(
        np.broadcast_to(
            np.arange(BW, dtype=np.float32).astype(ml_dtypes.bfloat16), (P, BW)
        )
    )
    shared = {
        "w1x": np.ascontiguousarray(W1[:D]),
        "w1a": np.ascontiguousarray(W1[D:]),
        "w2": np.ascontiguousarray(W2),
        "b1": np.ascontiguousarray(b1.reshape(D, 1)),
        "b2": np.ascontiguousarray(b2.reshape(D, 1)),
        "iota": iota,
    }
    in_maps = [{**pc, **shared} for pc in per_core]

    nc = _build_program(T, NB)
    trace = bool(int(os.environ.get("KBENCH_TRACE", "0")))
    LAST_RESULTS = run_bass_kernel_spmd(
        nc, in_maps, list(range(N_CORES)), trace=trace
    )
    results = LAST_RESULTS.results

    agg = np.empty((meta["n_pad"], D), np.float32)
    out = np.empty((meta["n_pad"], D), np.float32)
    for k in range(N_CORES):
        agg[k * nn_core : (k + 1) * nn_core] = results[k]["aggt"].T
        out[k * nn_core : (k + 1) * nn_core] = results[k]["outt"].T

    combined = np.concatenate([x, agg[:n_nodes]], axis=1)
    return out[:n_nodes], combined


# revision 16
# speedup vs baseline: 1.9370x; 1.0822x over previous
"""GNN message-passing (NodeModel) Trainium2 kernel.

Strategy ("shard nodes, bucket edges" — a refinement of the edge-sharding hint
that removes the all-reduce entirely):
  * Host buckets edges by destination-node bucket of BW=64 nodes (a counting
    sort by `row >> 6`).  Buckets are distributed contiguously over the 8
    cores, so each core owns a contiguous 1/8 slice of the node space and
    ALL edges that point into it.  No cross-core reduction is needed.
  * On device, each bucket's segment-sum runs on the tensor engine: for each
    128-edge tile, a one-hot "placement" matrix P[e, n] = (local_id[e] == n)
    is built with a DVE is_equal against an iota row, and
    aggT += attr_tile.T @ P accumulates in PSUM.  Padding edges carry local
    id -1 and contribute nothing.  BW=64 keeps the placement build (the DVE
    cost) at half of what 128-wide buckets would need.
  * Precision: edge attrs are split on the host into bf16 hi + lo parts
    (attr ≈ hi + lo to ~2^-16 relative).  The stationary operand packs
    [hi | lo] as 128 columns, so ONE bf16 matmul per edge tile produces
    both partial aggregates (PSUM rows 0-63 = hi, 64-127 = lo); an ACT copy
    plus DVE add folds them to fp32.  This halves tensor-engine time vs
    fp32 matmuls (which lower to 2 HW passes) at ~1e-5 relative accuracy.
  * The 2-layer MLP runs on the same core over its node slice, entirely in
    feature-major (transposed) layout:  hT = silu(W1x.T@xT + W1a.T@aggT + b1),
    outT = W2.T @ hT + b2  (fp32), batched 512 nodes per matmul.
  * Device outputs are feature-major [64, nodes/core]; the host transposes
    and assembles the full (out, combined) pair.
"""

import functools
import os
import sys

import numpy as np


def _ensure_path():
    try:
        import concourse  # noqa: F401
    except ImportError:
        for p in ("/opt/trn_rl_repo", "/root/.axon_site/_ro/trn_rl_repo"):
            if os.path.isdir(p):
                sys.path.insert(0, p)
                break


P = 128  # edge-tile size (contraction dim)
D = 64  # feature dim
BW = 128  # nodes per bucket (one-hot width)
N_CORES = 8

# Stash of the last BassKernelResults (for test harness introspection).
LAST_RESULTS = None


def _bf16_split(a):
    """Round-to-nearest-even split of fp32 `a` into bf16 hi/lo bit patterns."""
    u = a.view(np.uint32)
    hi_u = ((u + (((u >> 16) & 1) + 0x7FFF)) >> 16).astype(np.uint16)
    hi_f = (hi_u.astype(np.uint32) << 16).view(np.float32)
    r = np.asarray(a - hi_f, dtype=np.float32)
    ur = r.view(np.uint32)
    lo_u = ((ur + (((ur >> 16) & 1) + 0x7FFF)) >> 16).astype(np.uint16)
    return hi_u, lo_u


@functools.lru_cache(maxsize=None)
def _build_program(T: int, NB: int):
    """Build the Bass program.

    T  = edge tiles (of 128 edges) per node bucket
    NB = node buckets (of BW nodes) per core
    """
    _ensure_path()
    import concourse.tile as tile
    from concourse import bacc, mybir
    from contextlib import ExitStack

    f32 = mybir.dt.float32
    bf16 = mybir.dt.bfloat16
    NN = NB * BW  # nodes per core
    TC = T * P  # columns per bucket row in the hi|lo attr layout

    nc = bacc.Bacc("TRN2", target_bir_lowering=False, debug=False)

    attr_d = nc.declare_dram_parameter("attr", [NB * P, TC], bf16, isOutput=False)
    idx_d = nc.declare_dram_parameter("idx", [P, NB * T], bf16, isOutput=False)
    xt_d = nc.declare_dram_parameter("xt", [D, NN], f32, isOutput=False)
    w1x_d = nc.declare_dram_parameter("w1x", [D, D], f32, isOutput=False)
    w1a_d = nc.declare_dram_parameter("w1a", [D, D], f32, isOutput=False)
    w2_d = nc.declare_dram_parameter("w2", [D, D], f32, isOutput=False)
    b1_d = nc.declare_dram_parameter("b1", [D, 1], f32, isOutput=False)
    b2_d = nc.declare_dram_parameter("b2", [D, 1], f32, isOutput=False)
    iota_d = nc.declare_dram_parameter("iota", [P, BW], bf16, isOutput=False)
    aggt_o = nc.declare_dram_parameter("aggt", [D, NN], f32, isOutput=True)
    outt_o = nc.declare_dram_parameter("outt", [D, NN], f32, isOutput=True)

    with tile.TileContext(nc) as tc, ExitStack() as ctx:
        consts = ctx.enter_context(tc.tile_pool(name="consts", bufs=1))
        attr_pool = ctx.enter_context(tc.tile_pool(name="attr", bufs=6))
        plc_pool = ctx.enter_context(tc.tile_pool(name="plc", bufs=4))
        rep_pool = ctx.enter_context(tc.tile_pool(name="rep", bufs=4))
        sbout_pool = ctx.enter_context(tc.tile_pool(name="sbout", bufs=3))
        ps_agg = ctx.enter_context(tc.tile_pool(name="ps_agg", bufs=2, space="PSUM"))
        ps_mlp = ctx.enter_context(tc.tile_pool(name="ps_mlp", bufs=2, space="PSUM"))

        iota_sb = consts.tile([P, BW], bf16)
        nc.sync.dma_start(iota_sb[:], iota_d[:, :])
        b1_sb = consts.tile([D, 1], f32)
        nc.sync.dma_start(b1_sb[:], b1_d[:, :])
        b2_sb = consts.tile([D, 1], f32)
        nc.sync.dma_start(b2_sb[:], b2_d[:, :])
        idx_sb = consts.tile([P, NB * T], bf16)
        nc.sync.dma_start(idx_sb[:], idx_d[:, :])
        w1x_sb = consts.tile([D, D], f32)
        nc.sync.dma_start(w1x_sb[:], w1x_d[:, :])
        w1a_sb = consts.tile([D, D], f32)
        nc.sync.dma_start(w1a_sb[:], w1a_d[:, :])
        w2_sb = consts.tile([D, D], f32)
        nc.sync.dma_start(w2_sb[:], w2_d[:, :])
        xt_sb = consts.tile([D, NN], f32)
        nc.sync.dma_start(xt_sb[:], xt_d[:, :])

        # MLP is batched over groups of buckets (up to 512 nodes per matmul)
        G = 512 // BW
        groups = [list(range(g0, min(g0 + G, NB))) for g0 in range(0, NB, G)]
        for grp in groups:
            W = len(grp) * BW
            agg4 = sbout_pool.tile([D, G * BW], f32, tag="agg")
            for q, b in enumerate(grp):
                attr_sb = attr_pool.tile([P, TC], bf16)
                nc.sync.dma_start(attr_sb[:], attr_d[b * P : (b + 1) * P, :])

                # one batched op builds all T placement one-hots.  A
                # stride-0 (broadcast) operand forces the DVE TensorTensor
                # into 1x mode, so for 2/3 of buckets the ACT engine first
                # materializes idx_rep; the DVE compare then has unit-stride
                # bf16 operands and runs in 2x_1P mode.
                plc = plc_pool.tile([P, T, BW], bf16)
                idx_b = (
                    idx_sb[:, b * T : (b + 1) * T]
                    .unsqueeze(2)
                    .to_broadcast([P, T, BW])
                )
                iota_b = iota_sb[:].unsqueeze(1).to_broadcast([P, T, BW])
                if b % 3 == 0:  # mode A: direct 1x compare on DVE
                    nc.vector.tensor_tensor(
                        out=plc[:], in0=idx_b, in1=iota_b,
                        op=mybir.AluOpType.is_equal,
                    )
                else:  # mode B: ACT materializes the broadcast, DVE compares 2x
                    idx_rep = rep_pool.tile([P, T, BW], bf16)
                    nc.scalar.activation(
                        out=idx_rep[:], in_=idx_b,
                        func=mybir.ActivationFunctionType.Copy,
                    )
                    nc.vector.tensor_tensor(
                        out=plc[:], in0=idx_rep[:], in1=iota_b,
                        op=mybir.AluOpType.is_equal,
                    )

                agg_ps = ps_agg.tile([P, BW], f32)
                for t in range(T):
                    nc.tensor.matmul(
                        out=agg_ps[:],
                        lhsT=attr_sb[:, t * P : (t + 1) * P],
                        rhs=plc[:, t, :],
                        start=(t == 0),
                        stop=(t == T - 1),
                    )

                # fold hi (rows 0-63) + lo (rows 64-127) partial sums to fp32
                # (only one operand may be in PSUM: stage hi through ACT first)
                agg_hi = sbout_pool.tile([D, BW], f32, tag="agghi")
                nc.scalar.activation(
                    out=agg_hi[:], in_=agg_ps[0:D, :],
                    func=mybir.ActivationFunctionType.Copy,
                )
                nc.vector.tensor_add(
                    agg4[:, q * BW : (q + 1) * BW], agg_hi[:], agg_ps[D : 2 * D, :]
                )

            c0 = grp[0] * BW
            nc.sync.dma_start(aggt_o[:, c0 : c0 + W], agg4[:, :W])

            h_ps = ps_mlp.tile([D, G * BW], f32, tag="h")
            nc.tensor.matmul(
                out=h_ps[:, :W], lhsT=w1x_sb[:], rhs=xt_sb[:, c0 : c0 + W],
                start=True, stop=False,
            )
            nc.tensor.matmul(
                out=h_ps[:, :W], lhsT=w1a_sb[:], rhs=agg4[:, :W],
                start=False, stop=True,
            )
            h_sb = sbout_pool.tile([D, G * BW], f32, tag="hsb")
            nc.scalar.activation(
                out=h_sb[:, :W], in_=h_ps[:, :W],
                func=mybir.ActivationFunctionType.Silu,
                bias=b1_sb[:],
            )
            o_ps = ps_mlp.tile([D, G * BW], f32, tag="o")
            nc.tensor.matmul(
                out=o_ps[:, :W], lhsT=w2_sb[:], rhs=h_sb[:, :W],
                start=True, stop=True,
            )
            o_sb = sbout_pool.tile([D, G * BW], f32, tag="osb")
            nc.scalar.activation(
                out=o_sb[:, :W], in_=o_ps[:, :W],
                func=mybir.ActivationFunctionType.Identity,
                bias=b2_sb[:],
            )
            nc.sync.dma_start(outt_o[:, c0 : c0 + W], o_sb[:, :W])

    nc.compile()
    return nc


def _prepare(edge_index, edge_attr, x):
    """Host-side bucketing/sharding. Returns (in_maps_partial, meta)."""
    import ml_dtypes

    n_nodes = x.shape[0]
    n_edges = edge_index.shape[1]

    shift = BW.bit_length() - 1
    nb_total = -(-n_nodes // BW)  # buckets of BW nodes
    nb_total = -(-nb_total // N_CORES) * N_CORES  # round to multiple of n_cores
    NB = nb_total // N_CORES  # buckets per core
    n_pad = nb_total * BW

    row = np.asarray(edge_index[0], dtype=np.int64)
    bucket = row >> shift
    order = np.argsort(bucket, kind="stable")
    counts = np.bincount(bucket, minlength=nb_total)
    T = max(1, int(-(-counts.max() // P)))
    S = T * P

    starts = np.zeros(nb_total, np.int64)
    starts[1:] = np.cumsum(counts)[:-1]
    bs = bucket[order]
    dest = bs * S + (np.arange(n_edges, dtype=np.int64) - starts[bs])
    perm = np.full(nb_total * S, -1, np.int64)
    perm[dest] = order
    valid = perm >= 0
    perm_c = np.where(valid, perm, 0)

    attr_pad = np.ascontiguousarray(np.asarray(edge_attr, dtype=np.float32)[perm_c])
    hi_u, lo_u = _bf16_split(attr_pad)
    attr_hl = np.empty((nb_total * S, 2 * D), np.uint16)
    attr_hl[:, :D] = hi_u
    attr_hl[:, D:] = lo_u
    attr_dram = attr_hl.reshape(nb_total * P, T * P).view(ml_dtypes.bfloat16)

    idxl = np.where(
        valid, (row[perm_c] & (BW - 1)).astype(np.float32), np.float32(-1.0)
    )
    idxl = idxl.astype(ml_dtypes.bfloat16)  # ids 0..BW-1 and -1 are exact in bf16
    idx_bpt = idxl.reshape(nb_total, P, T)

    x_pad = np.zeros((n_pad, D), np.float32)
    x_pad[:n_nodes] = np.asarray(x, dtype=np.float32)

    nn_core = NB * BW
    per_core = []
    for k in range(N_CORES):
        r0, r1 = k * NB * P, (k + 1) * NB * P
        xt_k = np.ascontiguousarray(x_pad[k * nn_core : (k + 1) * nn_core].T)
        idx_k = np.ascontiguousarray(
            idx_bpt[k * NB : (k + 1) * NB].transpose(1, 0, 2).reshape(P, NB * T)
        )
        per_core.append({
            "attr": attr_dram[r0:r1],
            "idx": idx_k,
            "xt": xt_k,
        })
    meta = dict(T=T, NB=NB, n_pad=n_pad, nn_core=nn_core)
    return per_core, meta


def kernel(edge_index, edge_attr, x, W1, b1, W2, b2):
    global LAST_RESULTS
    _ensure_path()
    import ml_dtypes
    from concourse.bass_utils import run_bass_kernel_spmd

    edge_index = np.asarray(edge_index)
    edge_attr = np.asarray(edge_attr, dtype=np.float32)
    x = np.asarray(x, dtype=np.float32)
    W1 = np.asarray(W1, dtype=np.float32)
    b1 = np.asarray(b1, dtype=np.float32)
    W2 = np.asarray(W2, dtype=np.float32)
    b2 = np.asarray(b2, dtype=np.float32)

    n_nodes = x.shape[0]
    per_core, meta = _prepare(edge_index, edge_attr, x)
    T, NB, nn_core = meta["T"], meta["NB"], meta["nn_core"]

    iota = np.ascontiguousarray(
        np.broadcast_to(
            np.arange(BW, dtype=np.float32).astype(ml_dtypes.bfloat16), (P, BW)
        )
    )
    shared = {
        "w1x": np.ascontiguousarray(W1[:D]),
        "w1a": np.ascontiguousarray(W1[D:]),
        "w2": np.ascontiguousarray(W2),
        "b1": np.ascontiguousarray(b1.reshape(D, 1)),
        "b2": np.ascontiguousarray(b2.reshape(D, 1)),
        "iota": iota,
    }
    in_maps = [{**pc, **shared} for pc in per_core]

    nc = _build_program(T, NB)
    trace = bool(int(os.environ.get("KBENCH_TRACE", "0")))
    LAST_RESULTS = run_bass_kernel_spmd(
        nc, in_maps, list(range(N_CORES)), trace=trace
    )
    results = LAST_RESULTS.results

    agg = np.empty((meta["n_pad"], D), np.float32)
    out = np.empty((meta["n_pad"], D), np.float32)
    for k in range(N_CORES):
        agg[k * nn_core : (k + 1) * nn_core] = results[k]["aggt"].T
        out[k * nn_core : (k + 1) * nn_core] = results[k]["outt"].T

    combined = np.concatenate([x, agg[:n_nodes]], axis=1)
    return out[:n_nodes], combined


# revision 17
# speedup vs baseline: 1.9930x; 1.0289x over previous
"""GNN message-passing (NodeModel) Trainium2 kernel.

Strategy ("shard nodes, bucket edges" — a refinement of the edge-sharding hint
that removes the all-reduce entirely):
  * Host buckets edges by destination-node bucket of BW=64 nodes (a counting
    sort by `row >> 6`).  Buckets are distributed contiguously over the 8
    cores, so each core owns a contiguous 1/8 slice of the node space and
    ALL edges that point into it.  No cross-core reduction is needed.
  * On device, each bucket's segment-sum runs on the tensor engine: for each
    128-edge tile, a one-hot "placement" matrix P[e, n] = (local_id[e] == n)
    is built with a DVE is_equal against an iota row, and
    aggT += attr_tile.T @ P accumulates in PSUM.  Padding edges carry local
    id -1 and contribute nothing.  BW=64 keeps the placement build (the DVE
    cost) at half of what 128-wide buckets would need.
  * Precision: edge attrs are split on the host into bf16 hi + lo parts
    (attr ≈ hi + lo to ~2^-16 relative).  The stationary operand packs
    [hi | lo] as 128 columns, so ONE bf16 matmul per edge tile produces
    both partial aggregates (PSUM rows 0-63 = hi, 64-127 = lo); an ACT copy
    plus DVE add folds them to fp32.  This halves tensor-engine time vs
    fp32 matmuls (which lower to 2 HW passes) at ~1e-5 relative accuracy.
  * The 2-layer MLP runs on the same core over its node slice, entirely in
    feature-major (transposed) layout:  hT = silu(W1x.T@xT + W1a.T@aggT + b1),
    outT = W2.T @ hT + b2  (fp32), batched 512 nodes per matmul.
  * Device outputs are feature-major [64, nodes/core]; the host transposes
    and assembles the full (out, combined) pair.
"""

import functools
import os
import sys

import numpy as np


def _ensure_path():
    try:
        import concourse  # noqa: F401
    except ImportError:
        for p in ("/opt/trn_rl_repo", "/root/.axon_site/_ro/trn_rl_repo"):
            if os.path.isdir(p):
                sys.path.insert(0, p)
                break


P = 128  # edge-tile size (contraction dim)
D = 64  # feature dim
BW = 128  # nodes per bucket (one-hot width)
N_CORES = 8

# Stash of the last BassKernelResults (for test harness introspection).
LAST_RESULTS = None


def _bf16_split(a):
    """Round-to-nearest-even split of fp32 `a` into bf16 hi/lo bit patterns."""
    u = a.view(np.uint32)
    hi_u = ((u + (((u >> 16) & 1) + 0x7FFF)) >> 16).astype(np.uint16)
    hi_f = (hi_u.astype(np.uint32) << 16).view(np.float32)
    r = np.asarray(a - hi_f, dtype=np.float32)
    ur = r.view(np.uint32)
    lo_u = ((ur + (((ur >> 16) & 1) + 0x7FFF)) >> 16).astype(np.uint16)
    return hi_u, lo_u


@functools.lru_cache(maxsize=None)
def _build_program(T: int, NB: int):
    """Build the Bass program.

    T  = edge tiles (of 128 edges) per node bucket
    NB = node buckets (of BW nodes) per core
    """
    _ensure_path()
    import concourse.tile as tile
    from concourse import bacc, mybir
    from contextlib import ExitStack

    f32 = mybir.dt.float32
    bf16 = mybir.dt.bfloat16
    NN = NB * BW  # nodes per core
    TC = T * P  # columns per bucket row in the hi|lo attr layout

    nc = bacc.Bacc("TRN2", target_bir_lowering=False, debug=False)

    attr_d = nc.declare_dram_parameter("attr", [NB * P, TC], bf16, isOutput=False)
    idx_d = nc.declare_dram_parameter("idx", [P, NB * T], bf16, isOutput=False)
    xt_d = nc.declare_dram_parameter("xt", [D, NN], f32, isOutput=False)
    w1x_d = nc.declare_dram_parameter("w1x", [D, D], f32, isOutput=False)
    w1a_d = nc.declare_dram_parameter("w1a", [D, D], f32, isOutput=False)
    w2_d = nc.declare_dram_parameter("w2", [D, D], f32, isOutput=False)
    b1_d = nc.declare_dram_parameter("b1", [D, 1], f32, isOutput=False)
    b2_d = nc.declare_dram_parameter("b2", [D, 1], f32, isOutput=False)
    iota_d = nc.declare_dram_parameter("iota", [P, BW], bf16, isOutput=False)
    aggt_o = nc.declare_dram_parameter("aggt", [D, NN], f32, isOutput=True)
    outt_o = nc.declare_dram_parameter("outt", [D, NN], f32, isOutput=True)

    with tile.TileContext(nc) as tc, ExitStack() as ctx:
        consts = ctx.enter_context(tc.tile_pool(name="consts", bufs=1))
        attr_pool = ctx.enter_context(tc.tile_pool(name="attr", bufs=6))
        plc_pool = ctx.enter_context(tc.tile_pool(name="plc", bufs=4))
        rep_pool = ctx.enter_context(tc.tile_pool(name="rep", bufs=4))
        sbout_pool = ctx.enter_context(tc.tile_pool(name="sbout", bufs=3))
        ps_agg = ctx.enter_context(tc.tile_pool(name="ps_agg", bufs=2, space="PSUM"))
        ps_mlp = ctx.enter_context(tc.tile_pool(name="ps_mlp", bufs=2, space="PSUM"))

        iota_sb = consts.tile([P, BW], bf16)
        nc.sync.dma_start(iota_sb[:], iota_d[:, :])
        b1_sb = consts.tile([D, 1], f32)
        nc.sync.dma_start(b1_sb[:], b1_d[:, :])
        b2_sb = consts.tile([D, 1], f32)
        nc.sync.dma_start(b2_sb[:], b2_d[:, :])
        idx_sb = consts.tile([P, NB * T], bf16)
        nc.sync.dma_start(idx_sb[:], idx_d[:, :])
        w1x_sb = consts.tile([D, D], f32)
        nc.sync.dma_start(w1x_sb[:], w1x_d[:, :])
        w1a_sb = consts.tile([D, D], f32)
        nc.sync.dma_start(w1a_sb[:], w1a_d[:, :])
        w2_sb = consts.tile([D, D], f32)
        nc.sync.dma_start(w2_sb[:], w2_d[:, :])
        xt_sb = consts.tile([D, NN], f32)
        nc.sync.dma_start(xt_sb[:], xt_d[:, :])

        # MLP is batched over groups of buckets (up to 512 nodes per matmul)
        G = 512 // BW
        groups = [list(range(g0, min(g0 + G, NB))) for g0 in range(0, NB, G)]
        for grp in groups:
            W = len(grp) * BW
            agg4 = sbout_pool.tile([D, G * BW], f32, tag="agg")
            for q, b in enumerate(grp):
                attr_sb = attr_pool.tile([P, TC], bf16)
                nc.sync.dma_start(attr_sb[:], attr_d[b * P : (b + 1) * P, :])

                # one batched op builds all T placement one-hots.  A
                # stride-0 (broadcast) operand forces the DVE TensorTensor
                # into 1x mode, so for 2/3 of buckets the ACT engine first
                # materializes idx_rep; the DVE compare then has unit-stride
                # bf16 operands and runs in 2x_1P mode.
                plc = plc_pool.tile([P, T, BW], bf16)
                idx_b = (
                    idx_sb[:, b * T : (b + 1) * T]
                    .unsqueeze(2)
                    .to_broadcast([P, T, BW])
                )
                iota_b = iota_sb[:].unsqueeze(1).to_broadcast([P, T, BW])
                if b % 3 == 0:  # mode A: direct 1x compare on DVE
                    nc.vector.tensor_tensor(
                        out=plc[:], in0=idx_b, in1=iota_b,
                        op=mybir.AluOpType.is_equal,
                    )
                else:  # mode B: ACT materializes the broadcast, DVE compares 2x
                    idx_rep = rep_pool.tile([P, T, BW], bf16)
                    nc.scalar.activation(
                        out=idx_rep[:], in_=idx_b,
                        func=mybir.ActivationFunctionType.Copy,
                    )
                    nc.vector.tensor_tensor(
                        out=plc[:], in0=idx_rep[:], in1=iota_b,
                        op=mybir.AluOpType.is_equal,
                    )

                agg_ps = ps_agg.tile([P, BW], f32)
                for t in range(T):
                    nc.tensor.matmul(
                        out=agg_ps[:],
                        lhsT=attr_sb[:, t * P : (t + 1) * P],
                        rhs=plc[:, t, :],
                        start=(t == 0),
                        stop=(t == T - 1),
                    )

                # fold hi (rows 0-63) + lo (rows 64-127) partial sums to fp32
                # (only one operand may be in PSUM: stage hi through ACT first)
                agg_hi = sbout_pool.tile([D, BW], f32, tag="agghi")
                nc.scalar.activation(
                    out=agg_hi[:], in_=agg_ps[0:D, :],
                    func=mybir.ActivationFunctionType.Copy,
                )
                nc.vector.tensor_add(
                    agg4[:, q * BW : (q + 1) * BW], agg_hi[:], agg_ps[D : 2 * D, :]
                )

            c0 = grp[0] * BW
            nc.scalar.dma_start(aggt_o[:, c0 : c0 + W], agg4[:, :W])

            h_ps = ps_mlp.tile([D, G * BW], f32, tag="h")
            nc.tensor.matmul(
                out=h_ps[:, :W], lhsT=w1x_sb[:], rhs=xt_sb[:, c0 : c0 + W],
                start=True, stop=False,
            )
            nc.tensor.matmul(
                out=h_ps[:, :W], lhsT=w1a_sb[:], rhs=agg4[:, :W],
                start=False, stop=True,
            )
            h_sb = sbout_pool.tile([D, G * BW], f32, tag="hsb")
            nc.scalar.activation(
                out=h_sb[:, :W], in_=h_ps[:, :W],
                func=mybir.ActivationFunctionType.Silu,
                bias=b1_sb[:],
            )
            o_ps = ps_mlp.tile([D, G * BW], f32, tag="o")
            nc.tensor.matmul(
                out=o_ps[:, :W], lhsT=w2_sb[:], rhs=h_sb[:, :W],
                start=True, stop=True,
            )
            o_sb = sbout_pool.tile([D, G * BW], f32, tag="osb")
            nc.scalar.activation(
                out=o_sb[:, :W], in_=o_ps[:, :W],
                func=mybir.ActivationFunctionType.Identity,
                bias=b2_sb[:],
            )
            nc.scalar.dma_start(outt_o[:, c0 : c0 + W], o_sb[:, :W])

    nc.compile()
    return nc


def _prepare(edge_index, edge_attr, x):
    """Host-side bucketing/sharding. Returns (in_maps_partial, meta)."""
    import ml_dtypes

    n_nodes = x.shape[0]
    n_edges = edge_index.shape[1]

    shift = BW.bit_length() - 1
    nb_total = -(-n_nodes // BW)  # buckets of BW nodes
    nb_total = -(-nb_total // N_CORES) * N_CORES  # round to multiple of n_cores
    NB = nb_total // N_CORES  # buckets per core
    n_pad = nb_total * BW

    row = np.asarray(edge_index[0], dtype=np.int64)
    bucket = row >> shift
    order = np.argsort(bucket, kind="stable")
    counts = np.bincount(bucket, minlength=nb_total)
    T = max(1, int(-(-counts.max() // P)))
    S = T * P

    starts = np.zeros(nb_total, np.int64)
    starts[1:] = np.cumsum(counts)[:-1]
    bs = bucket[order]
    dest = bs * S + (np.arange(n_edges, dtype=np.int64) - starts[bs])
    perm = np.full(nb_total * S, -1, np.int64)
    perm[dest] = order
    valid = perm >= 0
    perm_c = np.where(valid, perm, 0)

    attr_pad = np.ascontiguousarray(np.asarray(edge_attr, dtype=np.float32)[perm_c])
    hi_u, lo_u = _bf16_split(attr_pad)
    attr_hl = np.empty((nb_total * S, 2 * D), np.uint16)
    attr_hl[:, :D] = hi_u
    attr_hl[:, D:] = lo_u
    attr_dram = attr_hl.reshape(nb_total * P, T * P).view(ml_dtypes.bfloat16)

    idxl = np.where(
        valid, (row[perm_c] & (BW - 1)).astype(np.float32), np.float32(-1.0)
    )
    idxl = idxl.astype(ml_dtypes.bfloat16)  # ids 0..BW-1 and -1 are exact in bf16
    idx_bpt = idxl.reshape(nb_total, P, T)

    x_pad = np.zeros((n_pad, D), np.float32)
    x_pad[:n_nodes] = np.asarray(x, dtype=np.float32)

    nn_core = NB * BW
    per_core = []
    for k in range(N_CORES):
        r0, r1 = k * NB * P, (k + 1) * NB * P
        xt_k = np.ascontiguousarray(x_pad[k * nn_core : (k + 1) * nn_core].T)
        idx_k = np.ascontiguousarray(
            idx_bpt[k * NB : (k + 1) * NB].transpose(1, 0, 2).reshape(P, NB * T)
        )
        per_core.append({
            "attr": attr_dram[r0:r1],
            "idx": idx_k,
            "xt": xt_k,
        })
    meta = dict(T=T, NB=NB, n_pad=n_pad, nn_core=nn_core)
    return per_core, meta


def kernel(edge_index, edge_attr, x, W1, b1, W2, b2):
    global LAST_RESULTS
    _ensure_path()
    import ml_dtypes
    from concourse.bass_utils import run_bass_kernel_spmd

    edge_index = np.asarray(edge_index)
    edge_attr = np.asarray(edge_attr, dtype=np.float32)
    x = np.asarray(x, dtype=np.float32)
    W1 = np.asarray(W1, dtype=np.float32)
    b1 = np.asarray(b1, dtype=np.float32)
    W2 = np.asarray(W2, dtype=np.float32)
    b2 = np.asarray(b2, dtype=np.float32)

    n_nodes = x.shape[0]
    per_core, meta = _prepare(edge_index, edge_attr, x)
    T, NB, nn_core = meta["T"], meta["NB"], meta["nn_core"]

    iota = np.ascontiguousarray(
        np.broadcast_to(
            np.arange(BW, dtype=np.float32).astype(ml_dtypes.bfloat16), (P, BW)
        )
    )
    shared = {
        "w1x": np.ascontiguousarray(W1[:D]),
        "w1a": np.ascontiguousarray(W1[D:]),
        "w2": np.ascontiguousarray(W2),
        "b1": np.ascontiguousarray(b1.reshape(D, 1)),
        "b2": np.ascontiguousarray(b2.reshape(D, 1)),
        "iota": iota,
    }
    in_maps = [{**pc, **shared} for pc in per_core]

    nc = _build_program(T, NB)
    trace = bool(int(os.environ.get("KBENCH_TRACE", "0")))
    LAST_RESULTS = run_bass_kernel_spmd(
        nc, in_maps, list(range(N_CORES)), trace=trace
    )
    results = LAST_RESULTS.results

    agg = np.empty((meta["n_pad"], D), np.float32)
    out = np.empty((meta["n_pad"], D), np.float32)
    for k in range(N_CORES):
        agg[k * nn_core : (k + 1) * nn_core] = results[k]["aggt"].T
        out[k * nn_core : (k + 1) * nn_core] = results[k]["outt"].T

    combined = np.concatenate([x, agg[:n_nodes]], axis=1)
    return out[:n_nodes], combined


# revision 18
# speedup vs baseline: 2.1549x; 1.0812x over previous
"""GNN message-passing (NodeModel) Trainium2 kernel.

Strategy ("shard nodes, bucket edges" — a refinement of the edge-sharding hint
that removes the all-reduce entirely):
  * Host buckets edges by destination-node bucket of BW=64 nodes (a counting
    sort by `row >> 6`).  Buckets are distributed contiguously over the 8
    cores, so each core owns a contiguous 1/8 slice of the node space and
    ALL edges that point into it.  No cross-core reduction is needed.
  * On device, each bucket's segment-sum runs on the tensor engine: for each
    128-edge tile, a one-hot "placement" matrix P[e, n] = (local_id[e] == n)
    is built with a DVE is_equal against an iota row, and
    aggT += attr_tile.T @ P accumulates in PSUM.  Padding edges carry local
    id -1 and contribute nothing.  BW=64 keeps the placement build (the DVE
    cost) at half of what 128-wide buckets would need.
  * Precision: edge attrs are split on the host into bf16 hi + lo parts
    (attr ≈ hi + lo to ~2^-16 relative).  The stationary operand packs
    [hi | lo] as 128 columns, so ONE bf16 matmul per edge tile produces
    both partial aggregates (PSUM rows 0-63 = hi, 64-127 = lo); an ACT copy
    plus DVE add folds them to fp32.  This halves tensor-engine time vs
    fp32 matmuls (which lower to 2 HW passes) at ~1e-5 relative accuracy.
  * The 2-layer MLP runs on the same core over its node slice, entirely in
    feature-major (transposed) layout:  hT = silu(W1x.T@xT + W1a.T@aggT + b1),
    outT = W2.T @ hT + b2  (fp32), batched 512 nodes per matmul.
  * Device outputs are feature-major [64, nodes/core]; the host transposes
    and assembles the full (out, combined) pair.
"""

import functools
import os
import sys

import numpy as np


def _ensure_path():
    try:
        import concourse  # noqa: F401
    except ImportError:
        for p in ("/opt/trn_rl_repo", "/root/.axon_site/_ro/trn_rl_repo"):
            if os.path.isdir(p):
                sys.path.insert(0, p)
                break


P = 128  # edge-tile size (contraction dim)
D = 64  # feature dim
BW = 128  # nodes per bucket (one-hot width)
N_CORES = 8

# Stash of the last BassKernelResults (for test harness introspection).
LAST_RESULTS = None


def _bf16_split(a):
    """Round-to-nearest-even split of fp32 `a` into bf16 hi/lo bit patterns."""
    u = a.view(np.uint32)
    hi_u = ((u + (((u >> 16) & 1) + 0x7FFF)) >> 16).astype(np.uint16)
    hi_f = (hi_u.astype(np.uint32) << 16).view(np.float32)
    r = np.asarray(a - hi_f, dtype=np.float32)
    ur = r.view(np.uint32)
    lo_u = ((ur + (((ur >> 16) & 1) + 0x7FFF)) >> 16).astype(np.uint16)
    return hi_u, lo_u


@functools.lru_cache(maxsize=None)
def _build_program(T: int, NB: int):
    """Build the Bass program.

    T  = edge tiles (of 128 edges) per node bucket
    NB = node buckets (of BW nodes) per core
    """
    _ensure_path()
    import concourse.tile as tile
    from concourse import bacc, mybir
    from contextlib import ExitStack

    f32 = mybir.dt.float32
    bf16 = mybir.dt.bfloat16
    NN = NB * BW  # nodes per core
    TC = T * P  # columns per bucket row in the hi|lo attr layout

    nc = bacc.Bacc("TRN2", target_bir_lowering=False, debug=False)

    attr_d = nc.declare_dram_parameter("attr", [NB * P, TC], bf16, isOutput=False)
    idx_d = nc.declare_dram_parameter("idx", [P, NB * T], bf16, isOutput=False)
    xt_d = nc.declare_dram_parameter("xt", [D, NN], f32, isOutput=False)
    w1x_d = nc.declare_dram_parameter("w1x", [D, D], f32, isOutput=False)
    w1a_d = nc.declare_dram_parameter("w1a", [D, D], f32, isOutput=False)
    w2_d = nc.declare_dram_parameter("w2", [D, D], f32, isOutput=False)
    b1_d = nc.declare_dram_parameter("b1", [D, 1], f32, isOutput=False)
    b2_d = nc.declare_dram_parameter("b2", [D, 1], f32, isOutput=False)
    iota_d = nc.declare_dram_parameter("iota", [P, BW], bf16, isOutput=False)
    aggt_o = nc.declare_dram_parameter("aggt", [D, NN], f32, isOutput=True)
    outt_o = nc.declare_dram_parameter("outt", [D, NN], f32, isOutput=True)

    with tile.TileContext(nc) as tc, ExitStack() as ctx:
        consts = ctx.enter_context(tc.tile_pool(name="consts", bufs=1))
        attr_pool = ctx.enter_context(tc.tile_pool(name="attr", bufs=6))
        plc_pool = ctx.enter_context(tc.tile_pool(name="plc", bufs=4))
        rep_pool = ctx.enter_context(tc.tile_pool(name="rep", bufs=4))
        sbout_pool = ctx.enter_context(tc.tile_pool(name="sbout", bufs=4))
        ps_agg = ctx.enter_context(tc.tile_pool(name="ps_agg", bufs=3, space="PSUM"))
        ps_mlp = ctx.enter_context(tc.tile_pool(name="ps_mlp", bufs=2, space="PSUM"))

        iota_sb = consts.tile([P, BW], bf16)
        nc.sync.dma_start(iota_sb[:], iota_d[:, :])
        b1_sb = consts.tile([D, 1], f32)
        nc.sync.dma_start(b1_sb[:], b1_d[:, :])
        b2_sb = consts.tile([D, 1], f32)
        nc.sync.dma_start(b2_sb[:], b2_d[:, :])
        idx_sb = consts.tile([P, NB * T], bf16)
        nc.sync.dma_start(idx_sb[:], idx_d[:, :])
        w1x_sb = consts.tile([D, D], f32)
        nc.sync.dma_start(w1x_sb[:], w1x_d[:, :])
        w1a_sb = consts.tile([D, D], f32)
        nc.sync.dma_start(w1a_sb[:], w1a_d[:, :])
        w2_sb = consts.tile([D, D], f32)
        nc.sync.dma_start(w2_sb[:], w2_d[:, :])
        xt_sb = consts.tile([D, NN], f32)
        nc.sync.dma_start(xt_sb[:], xt_d[:, :])

        # MLP is batched over groups of buckets (up to 512 nodes per matmul)
        G = 512 // BW
        groups = [list(range(g0, min(g0 + G, NB))) for g0 in range(0, NB, G)]
        for grp in groups:
            W = len(grp) * BW
            agg4 = sbout_pool.tile([D, G * BW], f32, tag="agg")
            for q, b in enumerate(grp):
                attr_sb = attr_pool.tile([P, TC], bf16)
                nc.sync.dma_start(attr_sb[:], attr_d[b * P : (b + 1) * P, :])

                # one batched op builds all T placement one-hots.  A
                # stride-0 (broadcast) operand forces the DVE TensorTensor
                # into 1x mode, so for 2/3 of buckets the ACT engine first
                # materializes idx_rep; the DVE compare then has unit-stride
                # bf16 operands and runs in 2x_1P mode.
                plc = plc_pool.tile([P, T, BW], bf16)
                idx_b = (
                    idx_sb[:, b * T : (b + 1) * T]
                    .unsqueeze(2)
                    .to_broadcast([P, T, BW])
                )
                iota_b = iota_sb[:].unsqueeze(1).to_broadcast([P, T, BW])
                if b % 3 == 0:  # mode A: direct 1x compare on DVE
                    nc.vector.tensor_tensor(
                        out=plc[:], in0=idx_b, in1=iota_b,
                        op=mybir.AluOpType.is_equal,
                    )
                else:  # mode B: ACT materializes the broadcast, DVE compares 2x
                    idx_rep = rep_pool.tile([P, T, BW], bf16)
                    nc.scalar.activation(
                        out=idx_rep[:], in_=idx_b,
                        func=mybir.ActivationFunctionType.Copy,
                    )
                    nc.vector.tensor_tensor(
                        out=plc[:], in0=idx_rep[:], in1=iota_b,
                        op=mybir.AluOpType.is_equal,
                    )

                agg_ps = ps_agg.tile([P, BW], f32)
                for t in range(T):
                    nc.tensor.matmul(
                        out=agg_ps[:],
                        lhsT=attr_sb[:, t * P : (t + 1) * P],
                        rhs=plc[:, t, :],
                        start=(t == 0),
                        stop=(t == T - 1),
                    )

                # fold hi (rows 0-63) + lo (rows 64-127) partial sums to fp32
                # (only one operand may be in PSUM: stage hi through ACT first)
                agg_hi = sbout_pool.tile([D, BW], f32, tag="agghi")
                nc.scalar.activation(
                    out=agg_hi[:], in_=agg_ps[0:D, :],
                    func=mybir.ActivationFunctionType.Copy,
                )
                nc.vector.tensor_add(
                    agg4[:, q * BW : (q + 1) * BW], agg_hi[:], agg_ps[D : 2 * D, :]
                )

            c0 = grp[0] * BW
            nc.scalar.dma_start(aggt_o[:, c0 : c0 + W], agg4[:, :W])

            h_ps = ps_mlp.tile([D, G * BW], f32, tag="h")
            nc.tensor.matmul(
                out=h_ps[:, :W], lhsT=w1x_sb[:], rhs=xt_sb[:, c0 : c0 + W],
                start=True, stop=False,
            )
            nc.tensor.matmul(
                out=h_ps[:, :W], lhsT=w1a_sb[:], rhs=agg4[:, :W],
                start=False, stop=True,
            )
            h_sb = sbout_pool.tile([D, G * BW], f32, tag="hsb")
            nc.scalar.activation(
                out=h_sb[:, :W], in_=h_ps[:, :W],
                func=mybir.ActivationFunctionType.Silu,
                bias=b1_sb[:],
            )
            o_ps = ps_mlp.tile([D, G * BW], f32, tag="o")
            nc.tensor.matmul(
                out=o_ps[:, :W], lhsT=w2_sb[:], rhs=h_sb[:, :W],
                start=True, stop=True,
            )
            o_sb = sbout_pool.tile([D, G * BW], f32, tag="osb")
            nc.scalar.activation(
                out=o_sb[:, :W], in_=o_ps[:, :W],
                func=mybir.ActivationFunctionType.Identity,
                bias=b2_sb[:],
            )
            nc.scalar.dma_start(outt_o[:, c0 : c0 + W], o_sb[:, :W])

    nc.compile()
    return nc


def _prepare(edge_index, edge_attr, x):
    """Host-side bucketing/sharding. Returns (in_maps_partial, meta)."""
    import ml_dtypes

    n_nodes = x.shape[0]
    n_edges = edge_index.shape[1]

    shift = BW.bit_length() - 1
    nb_total = -(-n_nodes // BW)  # buckets of BW nodes
    nb_total = -(-nb_total // N_CORES) * N_CORES  # round to multiple of n_cores
    NB = nb_total // N_CORES  # buckets per core
    n_pad = nb_total * BW

    row = np.asarray(edge_index[0], dtype=np.int64)
    bucket = row >> shift
    order = np.argsort(bucket, kind="stable")
    counts = np.bincount(bucket, minlength=nb_total)
    T = max(1, int(-(-counts.max() // P)))
    S = T * P

    starts = np.zeros(nb_total, np.int64)
    starts[1:] = np.cumsum(counts)[:-1]
    bs = bucket[order]
    dest = bs * S + (np.arange(n_edges, dtype=np.int64) - starts[bs])
    perm = np.full(nb_total * S, -1, np.int64)
    perm[dest] = order
    valid = perm >= 0
    perm_c = np.where(valid, perm, 0)

    attr_pad = np.ascontiguousarray(np.asarray(edge_attr, dtype=np.float32)[perm_c])
    hi_u, lo_u = _bf16_split(attr_pad)
    attr_hl = np.empty((nb_total * S, 2 * D), np.uint16)
    attr_hl[:, :D] = hi_u
    attr_hl[:, D:] = lo_u
    attr_dram = attr_hl.reshape(nb_total * P, T * P).view(ml_dtypes.bfloat16)

    idxl = np.where(
        valid, (row[perm_c] & (BW - 1)).astype(np.float32), np.float32(-1.0)
    )
    idxl = idxl.astype(ml_dtypes.bfloat16)  # ids 0..BW-1 and -1 are exact in bf16
    idx_bpt = idxl.reshape(nb_total, P, T)

    x_pad = np.zeros((n_pad, D), np.float32)
    x_pad[:n_nodes] = np.asarray(x, dtype=np.float32)

    nn_core = NB * BW
    per_core = []
    for k in range(N_CORES):
        r0, r1 = k * NB * P, (k + 1) * NB * P
        xt_k = np.ascontiguousarray(x_pad[k * nn_core : (k + 1) * nn_core].T)
        idx_k = np.ascontiguousarray(
            idx_bpt[k * NB : (k + 1) * NB].transpose(1, 0, 2).reshape(P, NB * T)
        )
        per_core.append({
            "attr": attr_dram[r0:r1],
            "idx": idx_k,
            "xt": xt_k,
        })
    meta = dict(T=T, NB=NB, n_pad=n_pad, nn_core=nn_core)
    return per_core, meta


def kernel(edge_index, edge_attr, x, W1, b1, W2, b2):
    global LAST_RESULTS
    _ensure_path()
    import ml_dtypes
    from concourse.bass_utils import run_bass_kernel_spmd

    edge_index = np.asarray(edge_index)
    edge_attr = np.asarray(edge_attr, dtype=np.float32)
    x = np.asarray(x, dtype=np.float32)
    W1 = np.asarray(W1, dtype=np.float32)
    b1 = np.asarray(b1, dtype=np.float32)
    W2 = np.asarray(W2, dtype=np.float32)
    b2 = np.asarray(b2, dtype=np.float32)

    n_nodes = x.shape[0]
    per_core, meta = _prepare(edge_index, edge_attr, x)
    T, NB, nn_core = meta["T"], meta["NB"], meta["nn_core"]

    iota = np.ascontiguousarray(
        np.broadcast_to(
            np.arange(BW, dtype=np.float32).astype(ml_dtypes.bfloat16), (P, BW)
        )
    )
    shared = {
        "w1x": np.ascontiguousarray(W1[:D]),
        "w1a": np.ascontiguousarray(W1[D:]),
        "w2": np.ascontiguousarray(W2),
        "b1": np.ascontiguousarray(b1.reshape(D, 1)),
        "b2": np.ascontiguousarray(b2.reshape(D, 1)),
        "iota": iota,
    }
    in_maps = [{**pc, **shared} for pc in per_core]

    nc = _build_program(T, NB)
    trace = bool(int(os.environ.get("KBENCH_TRACE", "0")))
    LAST_RESULTS = run_bass_kernel_spmd(
        nc, in_maps, list(range(N_CORES)), trace=trace
    )
    results = LAST_RESULTS.results

    agg = np.empty((meta["n_pad"], D), np.float32)
    out = np.empty((meta["n_pad"], D), np.float32)
    for k in range(N_CORES):
        agg[k * nn_core : (k + 1) * nn_core] = results[k]["aggt"].T
        out[k * nn_core : (k + 1) * nn_core] = results[k]["outt"].T

    combined = np.concatenate([x, agg[:n_nodes]], axis=1)
    return out[:n_nodes], combined


# revision 20
# speedup vs baseline: 2.1801x; 1.0117x over previous
"""GNN message-passing (NodeModel) Trainium2 kernel.

Strategy ("shard nodes, bucket edges" — a refinement of the edge-sharding hint
that removes the all-reduce entirely):
  * Host buckets edges by destination-node bucket of BW=64 nodes (a counting
    sort by `row >> 6`).  Buckets are distributed contiguously over the 8
    cores, so each core owns a contiguous 1/8 slice of the node space and
    ALL edges that point into it.  No cross-core reduction is needed.
  * On device, each bucket's segment-sum runs on the tensor engine: for each
    128-edge tile, a one-hot "placement" matrix P[e, n] = (local_id[e] == n)
    is built with a DVE is_equal against an iota row, and
    aggT += attr_tile.T @ P accumulates in PSUM.  Padding edges carry local
    id -1 and contribute nothing.  BW=64 keeps the placement build (the DVE
    cost) at half of what 128-wide buckets would need.
  * Precision: edge attrs are split on the host into bf16 hi + lo parts
    (attr ≈ hi + lo to ~2^-16 relative).  The stationary operand packs
    [hi | lo] as 128 columns, so ONE bf16 matmul per edge tile produces
    both partial aggregates (PSUM rows 0-63 = hi, 64-127 = lo); an ACT copy
    plus DVE add folds them to fp32.  This halves tensor-engine time vs
    fp32 matmuls (which lower to 2 HW passes) at ~1e-5 relative accuracy.
  * The 2-layer MLP runs on the same core over its node slice, entirely in
    feature-major (transposed) layout:  hT = silu(W1x.T@xT + W1a.T@aggT + b1),
    outT = W2.T @ hT + b2  (fp32), batched 512 nodes per matmul.
  * Device outputs are feature-major [64, nodes/core]; the host transposes
    and assembles the full (out, combined) pair.
"""

import functools
import os
import sys

import numpy as np


def _ensure_path():
    try:
        import concourse  # noqa: F401
    except ImportError:
        for p in ("/opt/trn_rl_repo", "/root/.axon_site/_ro/trn_rl_repo"):
            if os.path.isdir(p):
                sys.path.insert(0, p)
                break


P = 128  # edge-tile size (contraction dim)
D = 64  # feature dim
BW = 128  # nodes per bucket (one-hot width)
N_CORES = 8

# Stash of the last BassKernelResults (for test harness introspection).
LAST_RESULTS = None


def _bf16_split(a):
    """Round-to-nearest-even split of fp32 `a` into bf16 hi/lo bit patterns."""
    u = a.view(np.uint32)
    hi_u = ((u + (((u >> 16) & 1) + 0x7FFF)) >> 16).astype(np.uint16)
    hi_f = (hi_u.astype(np.uint32) << 16).view(np.float32)
    r = np.asarray(a - hi_f, dtype=np.float32)
    ur = r.view(np.uint32)
    lo_u = ((ur + (((ur >> 16) & 1) + 0x7FFF)) >> 16).astype(np.uint16)
    return hi_u, lo_u


@functools.lru_cache(maxsize=None)
def _build_program(T_LIST: tuple, NB: int):
    """Build the Bass program.

    T_LIST = per-position edge-tile counts (buckets are sorted by edge count
             on the host so one descending tile-count pattern fits all cores)
    NB     = node buckets (of BW nodes) per core
    """
    _ensure_path()
    import concourse.tile as tile
    from concourse import bacc, mybir
    from contextlib import ExitStack

    f32 = mybir.dt.float32
    bf16 = mybir.dt.bfloat16
    NN = NB * BW  # nodes per core
    ST = sum(T_LIST)
    SOFF = [0] * NB  # per-position tile offsets
    for j in range(1, NB):
        SOFF[j] = SOFF[j - 1] + T_LIST[j - 1]
    Tmax = max(T_LIST)

    nc = bacc.Bacc("TRN2", target_bir_lowering=False, debug=False)

    attr_d = nc.declare_dram_parameter("attr", [P * ST * P], bf16, isOutput=False)
    idx_d = nc.declare_dram_parameter("idx", [P, ST], bf16, isOutput=False)
    xt_d = nc.declare_dram_parameter("xt", [D, NN], f32, isOutput=False)
    w1x_d = nc.declare_dram_parameter("w1x", [D, D], f32, isOutput=False)
    w1a_d = nc.declare_dram_parameter("w1a", [D, D], f32, isOutput=False)
    w2_d = nc.declare_dram_parameter("w2", [D, D], f32, isOutput=False)
    b1_d = nc.declare_dram_parameter("b1", [D, 1], f32, isOutput=False)
    b2_d = nc.declare_dram_parameter("b2", [D, 1], f32, isOutput=False)
    iota_d = nc.declare_dram_parameter("iota", [P, BW], bf16, isOutput=False)
    aggt_o = nc.declare_dram_parameter("aggt", [D, NN], f32, isOutput=True)
    outt_o = nc.declare_dram_parameter("outt", [D, NN], f32, isOutput=True)

    with tile.TileContext(nc) as tc, ExitStack() as ctx:
        consts = ctx.enter_context(tc.tile_pool(name="consts", bufs=1))
        attr_pool = ctx.enter_context(tc.tile_pool(name="attr", bufs=6))
        plc_pool = ctx.enter_context(tc.tile_pool(name="plc", bufs=4))
        rep_pool = ctx.enter_context(tc.tile_pool(name="rep", bufs=4))
        sbout_pool = ctx.enter_context(tc.tile_pool(name="sbout", bufs=4))
        ps_agg = ctx.enter_context(tc.tile_pool(name="ps_agg", bufs=3, space="PSUM"))
        ps_mlp = ctx.enter_context(tc.tile_pool(name="ps_mlp", bufs=2, space="PSUM"))

        iota_sb = consts.tile([P, BW], bf16)
        nc.sync.dma_start(iota_sb[:], iota_d[:, :])
        b1_sb = consts.tile([D, 1], f32)
        nc.sync.dma_start(b1_sb[:], b1_d[:, :])
        b2_sb = consts.tile([D, 1], f32)
        nc.sync.dma_start(b2_sb[:], b2_d[:, :])
        idx_sb = consts.tile([P, ST], bf16)
        nc.sync.dma_start(idx_sb[:], idx_d[:, :])
        w1x_sb = consts.tile([D, D], f32)
        nc.sync.dma_start(w1x_sb[:], w1x_d[:, :])
        w1a_sb = consts.tile([D, D], f32)
        nc.sync.dma_start(w1a_sb[:], w1a_d[:, :])
        w2_sb = consts.tile([D, D], f32)
        nc.sync.dma_start(w2_sb[:], w2_d[:, :])
        xt_sb = consts.tile([D, NN], f32)
        nc.sync.dma_start(xt_sb[:], xt_d[:, :])

        # MLP is batched over groups of buckets (up to 512 nodes per matmul)
        G = 512 // BW
        groups = [list(range(g0, min(g0 + G, NB))) for g0 in range(0, NB, G)]
        for grp in groups:
            W = len(grp) * BW
            agg4 = sbout_pool.tile([D, G * BW], f32, tag="agg")
            for q, b in enumerate(grp):
                Tj = T_LIST[b]
                o0 = P * P * SOFF[b]
                attr_sb = attr_pool.tile([P, Tmax * P], bf16)
                nc.sync.dma_start(
                    attr_sb[:, : Tj * P],
                    attr_d[o0 : o0 + P * Tj * P].rearrange("(p c) -> p c", p=P),
                )

                # one batched op builds all Tj placement one-hots.  A
                # stride-0 (broadcast) operand forces the DVE TensorTensor
                # into 1x mode, so for 2/3 of buckets the ACT engine first
                # materializes idx_rep; the DVE compare then has unit-stride
                # bf16 operands and runs in 2x_1P mode.
                plc = plc_pool.tile([P, Tmax, BW], bf16)
                idx_b = (
                    idx_sb[:, SOFF[b] : SOFF[b] + Tj]
                    .unsqueeze(2)
                    .to_broadcast([P, Tj, BW])
                )
                iota_b = iota_sb[:].unsqueeze(1).to_broadcast([P, Tj, BW])
                if b % 3 == 0:  # mode A: direct 1x compare on DVE
                    nc.vector.tensor_tensor(
                        out=plc[:, :Tj, :], in0=idx_b, in1=iota_b,
                        op=mybir.AluOpType.is_equal,
                    )
                else:  # mode B: ACT materializes the broadcast, DVE compares 2x
                    idx_rep = rep_pool.tile([P, Tmax, BW], bf16)
                    nc.scalar.activation(
                        out=idx_rep[:, :Tj, :], in_=idx_b,
                        func=mybir.ActivationFunctionType.Copy,
                    )
                    nc.vector.tensor_tensor(
                        out=plc[:, :Tj, :], in0=idx_rep[:, :Tj, :], in1=iota_b,
                        op=mybir.AluOpType.is_equal,
                    )

                agg_ps = ps_agg.tile([P, BW], f32)
                for t in range(Tj):
                    nc.tensor.matmul(
                        out=agg_ps[:],
                        lhsT=attr_sb[:, t * P : (t + 1) * P],
                        rhs=plc[:, t, :],
                        start=(t == 0),
                        stop=(t == Tj - 1),
                    )

                # fold hi (rows 0-63) + lo (rows 64-127) partial sums to fp32
                # (only one operand may be in PSUM: stage hi through ACT first)
                agg_hi = sbout_pool.tile([D, BW], f32, tag="agghi")
                nc.scalar.activation(
                    out=agg_hi[:], in_=agg_ps[0:D, :],
                    func=mybir.ActivationFunctionType.Copy,
                )
                nc.vector.tensor_add(
                    agg4[:, q * BW : (q + 1) * BW], agg_hi[:], agg_ps[D : 2 * D, :]
                )

            c0 = grp[0] * BW
            nc.scalar.dma_start(aggt_o[:, c0 : c0 + W], agg4[:, :W])

            h_ps = ps_mlp.tile([D, G * BW], f32, tag="h")
            nc.tensor.matmul(
                out=h_ps[:, :W], lhsT=w1x_sb[:], rhs=xt_sb[:, c0 : c0 + W],
                start=True, stop=False,
            )
            nc.tensor.matmul(
                out=h_ps[:, :W], lhsT=w1a_sb[:], rhs=agg4[:, :W],
                start=False, stop=True,
            )
            h_sb = sbout_pool.tile([D, G * BW], f32, tag="hsb")
            nc.scalar.activation(
                out=h_sb[:, :W], in_=h_ps[:, :W],
                func=mybir.ActivationFunctionType.Silu,
                bias=b1_sb[:],
            )
            o_ps = ps_mlp.tile([D, G * BW], f32, tag="o")
            nc.tensor.matmul(
                out=o_ps[:, :W], lhsT=w2_sb[:], rhs=h_sb[:, :W],
                start=True, stop=True,
            )
            o_sb = sbout_pool.tile([D, G * BW], f32, tag="osb")
            nc.scalar.activation(
                out=o_sb[:, :W], in_=o_ps[:, :W],
                func=mybir.ActivationFunctionType.Identity,
                bias=b2_sb[:],
            )
            nc.scalar.dma_start(outt_o[:, c0 : c0 + W], o_sb[:, :W])

    nc.compile()
    return nc


def _prepare(edge_index, edge_attr, x):
    """Host-side bucketing/sharding. Returns (in_maps_partial, meta)."""
    import ml_dtypes

    n_nodes = x.shape[0]
    n_edges = edge_index.shape[1]

    shift = BW.bit_length() - 1
    nb_total = -(-n_nodes // BW)  # buckets of BW nodes
    nb_total = -(-nb_total // N_CORES) * N_CORES  # round to multiple of n_cores
    NB = nb_total // N_CORES  # buckets per core
    n_pad = nb_total * BW

    row = np.asarray(edge_index[0], dtype=np.int64)
    bucket = row >> shift
    order = np.argsort(bucket, kind="stable")
    counts = np.bincount(bucket, minlength=nb_total)

    # Sort each core's buckets by edge count (descending) so that one
    # tile-count pattern T_LIST (the per-position max over cores) fits all
    # cores with minimal padding; outputs are un-permuted on the host.
    counts_pc = counts.reshape(N_CORES, NB)
    perm_pc = np.argsort(-counts_pc, kind="stable")  # [cores, NB] pos -> local bkt
    sorted_counts = np.take_along_axis(counts_pc, perm_pc, axis=1)
    T_arr = np.maximum(1, -(-sorted_counts.max(axis=0) // P))  # [NB]
    T_LIST = tuple(int(t) for t in T_arr)
    ST = int(T_arr.sum())
    # per-position edge offsets within a core's padded edge array
    pos_edge_off = np.zeros(NB, np.int64)
    pos_edge_off[1:] = np.cumsum(T_arr[:-1] * P)
    e_core_pad = int(ST * P)

    # rank of each global bucket within its core's order
    rank = np.empty(nb_total, np.int64)
    core_idx = np.repeat(np.arange(N_CORES), NB)
    rank[(np.arange(nb_total) // NB) * NB + perm_pc.ravel()] = np.tile(
        np.arange(NB), N_CORES
    )

    starts = np.zeros(nb_total, np.int64)
    starts[1:] = np.cumsum(counts)[:-1]
    bs = bucket[order]
    dest = (
        (bs // NB) * e_core_pad
        + pos_edge_off[rank[bs]]
        + (np.arange(n_edges, dtype=np.int64) - starts[bs])
    )
    perm = np.full(N_CORES * e_core_pad, -1, np.int64)
    perm[dest] = order
    valid = perm >= 0
    perm_c = np.where(valid, perm, 0)

    attr_pad = np.ascontiguousarray(np.asarray(edge_attr, dtype=np.float32)[perm_c])
    hi_u, lo_u = _bf16_split(attr_pad)
    attr_hl = np.empty((N_CORES * e_core_pad, 2 * D), np.uint16)
    attr_hl[:, :D] = hi_u
    attr_hl[:, D:] = lo_u
    attr_flat = attr_hl.reshape(N_CORES, e_core_pad * 2 * D).view(ml_dtypes.bfloat16)

    idxl = np.where(
        valid, (row[perm_c] & (BW - 1)).astype(np.float32), np.float32(-1.0)
    ).astype(ml_dtypes.bfloat16)  # ids 0..BW-1 and -1 are exact in bf16
    idxl = idxl.reshape(N_CORES, e_core_pad)

    x_pad = np.zeros((n_pad, D), np.float32)
    x_pad[:n_nodes] = np.asarray(x, dtype=np.float32)
    x_blocks = x_pad.reshape(nb_total, BW, D)

    nn_core = NB * BW
    per_core = []
    for k in range(N_CORES):
        sel = k * NB + perm_pc[k]
        xt_k = np.ascontiguousarray(
            x_blocks[sel].transpose(2, 0, 1).reshape(D, nn_core)
        )
        # idx: [P, ST] with position j's [P, T_j] block at column SOFF[j]
        idx_k = np.empty((P, ST), ml_dtypes.bfloat16)
        off = 0
        for j in range(NB):
            tj = int(T_arr[j])
            blk = idxl[k, pos_edge_off[j] : pos_edge_off[j] + tj * P]
            idx_k[:, off : off + tj] = blk.reshape(P, tj)
            off += tj
        per_core.append({
            "attr": attr_flat[k],
            "idx": np.ascontiguousarray(idx_k),
            "xt": xt_k,
        })
    meta = dict(
        T_LIST=T_LIST, NB=NB, n_pad=n_pad, nn_core=nn_core, perm_pc=perm_pc
    )
    return per_core, meta


def kernel(edge_index, edge_attr, x, W1, b1, W2, b2):
    global LAST_RESULTS
    _ensure_path()
    import ml_dtypes
    from concourse.bass_utils import run_bass_kernel_spmd

    edge_index = np.asarray(edge_index)
    edge_attr = np.asarray(edge_attr, dtype=np.float32)
    x = np.asarray(x, dtype=np.float32)
    W1 = np.asarray(W1, dtype=np.float32)
    b1 = np.asarray(b1, dtype=np.float32)
    W2 = np.asarray(W2, dtype=np.float32)
    b2 = np.asarray(b2, dtype=np.float32)

    n_nodes = x.shape[0]
    per_core, meta = _prepare(edge_index, edge_attr, x)
    T_LIST, NB, nn_core = meta["T_LIST"], meta["NB"], meta["nn_core"]

    iota = np.ascontiguousarray(
        np.broadcast_to(
            np.arange(BW, dtype=np.float32).astype(ml_dtypes.bfloat16), (P, BW)
        )
    )
    shared = {
        "w1x": np.ascontiguousarray(W1[:D]),
        "w1a": np.ascontiguousarray(W1[D:]),
        "w2": np.ascontiguousarray(W2),
        "b1": np.ascontiguousarray(b1.reshape(D, 1)),
        "b2": np.ascontiguousarray(b2.reshape(D, 1)),
        "iota": iota,
    }
    in_maps = [{**pc, **shared} for pc in per_core]

    nc = _build_program(T_LIST, NB)
    trace = bool(int(os.environ.get("KBENCH_TRACE", "0")))
    LAST_RESULTS = run_bass_kernel_spmd(
        nc, in_maps, list(range(N_CORES)), trace=trace
    )
    results = LAST_RESULTS.results

    perm_pc = meta["perm_pc"]
    agg = np.empty((meta["n_pad"] // BW, BW, D), np.float32)
    out = np.empty_like(agg)
    for k in range(N_CORES):
        sel = k * NB + perm_pc[k]
        agg[sel] = results[k]["aggt"].T.reshape(NB, BW, D)
        out[sel] = results[k]["outt"].T.reshape(NB, BW, D)
    agg = agg.reshape(-1, D)
    out = out.reshape(-1, D)

    combined = np.concatenate([x, agg[:n_nodes]], axis=1)
    return out[:n_nodes], combined


# revision 21
# speedup vs baseline: 2.1928x; 1.0058x over previous
"""GNN message-passing (NodeModel) Trainium2 kernel.

Strategy ("shard nodes, bucket edges" — a refinement of the edge-sharding hint
that removes the all-reduce entirely):
  * Host buckets edges by destination-node bucket of BW=64 nodes (a counting
    sort by `row >> 6`).  Buckets are distributed contiguously over the 8
    cores, so each core owns a contiguous 1/8 slice of the node space and
    ALL edges that point into it.  No cross-core reduction is needed.
  * On device, each bucket's segment-sum runs on the tensor engine: for each
    128-edge tile, a one-hot "placement" matrix P[e, n] = (local_id[e] == n)
    is built with a DVE is_equal against an iota row, and
    aggT += attr_tile.T @ P accumulates in PSUM.  Padding edges carry local
    id -1 and contribute nothing.  BW=64 keeps the placement build (the DVE
    cost) at half of what 128-wide buckets would need.
  * Precision: edge attrs are split on the host into bf16 hi + lo parts
    (attr ≈ hi + lo to ~2^-16 relative).  The stationary operand packs
    [hi | lo] as 128 columns, so ONE bf16 matmul per edge tile produces
    both partial aggregates (PSUM rows 0-63 = hi, 64-127 = lo); an ACT copy
    plus DVE add folds them to fp32.  This halves tensor-engine time vs
    fp32 matmuls (which lower to 2 HW passes) at ~1e-5 relative accuracy.
  * The 2-layer MLP runs on the same core over its node slice, entirely in
    feature-major (transposed) layout:  hT = silu(W1x.T@xT + W1a.T@aggT + b1),
    outT = W2.T @ hT + b2  (fp32), batched 512 nodes per matmul.
  * Device outputs are feature-major [64, nodes/core]; the host transposes
    and assembles the full (out, combined) pair.
"""

import functools
import os
import sys

import numpy as np


def _ensure_path():
    try:
        import concourse  # noqa: F401
    except ImportError:
        for p in ("/opt/trn_rl_repo", "/root/.axon_site/_ro/trn_rl_repo"):
            if os.path.isdir(p):
                sys.path.insert(0, p)
                break


P = 128  # edge-tile size (contraction dim)
D = 64  # feature dim
BW = 128  # nodes per bucket (one-hot width)
N_CORES = 8

# Stash of the last BassKernelResults (for test harness introspection).
LAST_RESULTS = None


def _bf16_split(a):
    """Round-to-nearest-even split of fp32 `a` into bf16 hi/lo bit patterns."""
    u = a.view(np.uint32)
    hi_u = ((u + (((u >> 16) & 1) + 0x7FFF)) >> 16).astype(np.uint16)
    hi_f = (hi_u.astype(np.uint32) << 16).view(np.float32)
    r = np.asarray(a - hi_f, dtype=np.float32)
    ur = r.view(np.uint32)
    lo_u = ((ur + (((ur >> 16) & 1) + 0x7FFF)) >> 16).astype(np.uint16)
    return hi_u, lo_u


@functools.lru_cache(maxsize=None)
def _build_program(T_LIST: tuple, NB: int):
    """Build the Bass program.

    T_LIST = per-position edge-tile counts (buckets are sorted by edge count
             on the host so one descending tile-count pattern fits all cores)
    NB     = node buckets (of BW nodes) per core
    """
    _ensure_path()
    import concourse.tile as tile
    from concourse import bacc, mybir
    from contextlib import ExitStack

    f32 = mybir.dt.float32
    bf16 = mybir.dt.bfloat16
    NN = NB * BW  # nodes per core
    ST = sum(T_LIST)
    SOFF = [0] * NB  # per-position tile offsets
    for j in range(1, NB):
        SOFF[j] = SOFF[j - 1] + T_LIST[j - 1]
    Tmax = max(T_LIST)

    nc = bacc.Bacc("TRN2", target_bir_lowering=False, debug=False)

    attr_d = nc.declare_dram_parameter("attr", [P * ST * P], bf16, isOutput=False)
    idx_d = nc.declare_dram_parameter("idx", [P, ST], bf16, isOutput=False)
    xt_d = nc.declare_dram_parameter("xt", [D, NN], f32, isOutput=False)
    w1x_d = nc.declare_dram_parameter("w1x", [D, D], f32, isOutput=False)
    w1a_d = nc.declare_dram_parameter("w1a", [D, D], f32, isOutput=False)
    w2_d = nc.declare_dram_parameter("w2", [D, D], f32, isOutput=False)
    b1_d = nc.declare_dram_parameter("b1", [D, 1], f32, isOutput=False)
    b2_d = nc.declare_dram_parameter("b2", [D, 1], f32, isOutput=False)
    iota_d = nc.declare_dram_parameter("iota", [P, BW], bf16, isOutput=False)
    aggt_o = nc.declare_dram_parameter("aggt", [D, NN], f32, isOutput=True)
    outt_o = nc.declare_dram_parameter("outt", [D, NN], f32, isOutput=True)

    with tile.TileContext(nc) as tc, ExitStack() as ctx:
        consts = ctx.enter_context(tc.tile_pool(name="consts", bufs=1))
        attr_pool = ctx.enter_context(tc.tile_pool(name="attr", bufs=8))
        plc_pool = ctx.enter_context(tc.tile_pool(name="plc", bufs=4))
        rep_pool = ctx.enter_context(tc.tile_pool(name="rep", bufs=4))
        sbout_pool = ctx.enter_context(tc.tile_pool(name="sbout", bufs=4))
        ps_agg = ctx.enter_context(tc.tile_pool(name="ps_agg", bufs=3, space="PSUM"))
        ps_mlp = ctx.enter_context(tc.tile_pool(name="ps_mlp", bufs=2, space="PSUM"))

        iota_sb = consts.tile([P, BW], bf16)
        nc.scalar.dma_start(iota_sb[:], iota_d[:, :])
        b1_sb = consts.tile([D, 1], f32)
        nc.scalar.dma_start(b1_sb[:], b1_d[:, :])
        b2_sb = consts.tile([D, 1], f32)
        nc.scalar.dma_start(b2_sb[:], b2_d[:, :])
        idx_sb = consts.tile([P, ST], bf16)
        nc.scalar.dma_start(idx_sb[:], idx_d[:, :])
        w1x_sb = consts.tile([D, D], f32)
        nc.scalar.dma_start(w1x_sb[:], w1x_d[:, :])
        w1a_sb = consts.tile([D, D], f32)
        nc.scalar.dma_start(w1a_sb[:], w1a_d[:, :])
        w2_sb = consts.tile([D, D], f32)
        nc.scalar.dma_start(w2_sb[:], w2_d[:, :])
        xt_sb = consts.tile([D, NN], f32)
        nc.scalar.dma_start(xt_sb[:], xt_d[:, :])

        # MLP is batched over groups of buckets (up to 512 nodes per matmul)
        G = 512 // BW
        groups = [list(range(g0, min(g0 + G, NB))) for g0 in range(0, NB, G)]
        for grp in groups:
            W = len(grp) * BW
            agg4 = sbout_pool.tile([D, G * BW], f32, tag="agg")
            for q, b in enumerate(grp):
                Tj = T_LIST[b]
                o0 = P * P * SOFF[b]
                attr_sb = attr_pool.tile([P, Tmax * P], bf16)
                nc.sync.dma_start(
                    attr_sb[:, : Tj * P],
                    attr_d[o0 : o0 + P * Tj * P].rearrange("(p c) -> p c", p=P),
                )

                # one batched op builds all Tj placement one-hots.  A
                # stride-0 (broadcast) operand forces the DVE TensorTensor
                # into 1x mode, so for 2/3 of buckets the ACT engine first
                # materializes idx_rep; the DVE compare then has unit-stride
                # bf16 operands and runs in 2x_1P mode.
                plc = plc_pool.tile([P, Tmax, BW], bf16)
                idx_b = (
                    idx_sb[:, SOFF[b] : SOFF[b] + Tj]
                    .unsqueeze(2)
                    .to_broadcast([P, Tj, BW])
                )
                iota_b = iota_sb[:].unsqueeze(1).to_broadcast([P, Tj, BW])
                if b % 3 == 0:  # mode A: direct 1x compare on DVE
                    nc.vector.tensor_tensor(
                        out=plc[:, :Tj, :], in0=idx_b, in1=iota_b,
                        op=mybir.AluOpType.is_equal,
                    )
                else:  # mode B: ACT materializes the broadcast, DVE compares 2x
                    idx_rep = rep_pool.tile([P, Tmax, BW], bf16)
                    nc.scalar.activation(
                        out=idx_rep[:, :Tj, :], in_=idx_b,
                        func=mybir.ActivationFunctionType.Copy,
                    )
                    nc.vector.tensor_tensor(
                        out=plc[:, :Tj, :], in0=idx_rep[:, :Tj, :], in1=iota_b,
                        op=mybir.AluOpType.is_equal,
                    )

                agg_ps = ps_agg.tile([P, BW], f32)
                for t in range(Tj):
                    nc.tensor.matmul(
                        out=agg_ps[:],
                        lhsT=attr_sb[:, t * P : (t + 1) * P],
                        rhs=plc[:, t, :],
                        start=(t == 0),
                        stop=(t == Tj - 1),
                    )

                # fold hi (rows 0-63) + lo (rows 64-127) partial sums to fp32
                # (only one operand may be in PSUM: stage hi through ACT first)
                agg_hi = sbout_pool.tile([D, BW], f32, tag="agghi")
                nc.scalar.activation(
                    out=agg_hi[:], in_=agg_ps[0:D, :],
                    func=mybir.ActivationFunctionType.Copy,
                )
                nc.vector.tensor_add(
                    agg4[:, q * BW : (q + 1) * BW], agg_hi[:], agg_ps[D : 2 * D, :]
                )

            c0 = grp[0] * BW
            nc.scalar.dma_start(aggt_o[:, c0 : c0 + W], agg4[:, :W])

            h_ps = ps_mlp.tile([D, G * BW], f32, tag="h")
            nc.tensor.matmul(
                out=h_ps[:, :W], lhsT=w1x_sb[:], rhs=xt_sb[:, c0 : c0 + W],
                start=True, stop=False,
            )
            nc.tensor.matmul(
                out=h_ps[:, :W], lhsT=w1a_sb[:], rhs=agg4[:, :W],
                start=False, stop=True,
            )
            h_sb = sbout_pool.tile([D, G * BW], f32, tag="hsb")
            nc.scalar.activation(
                out=h_sb[:, :W], in_=h_ps[:, :W],
                func=mybir.ActivationFunctionType.Silu,
                bias=b1_sb[:],
            )
            o_ps = ps_mlp.tile([D, G * BW], f32, tag="o")
            nc.tensor.matmul(
                out=o_ps[:, :W], lhsT=w2_sb[:], rhs=h_sb[:, :W],
                start=True, stop=True,
            )
            o_sb = sbout_pool.tile([D, G * BW], f32, tag="osb")
            nc.scalar.activation(
                out=o_sb[:, :W], in_=o_ps[:, :W],
                func=mybir.ActivationFunctionType.Identity,
                bias=b2_sb[:],
            )
            nc.scalar.dma_start(outt_o[:, c0 : c0 + W], o_sb[:, :W])

    nc.compile()
    return nc


def _prepare(edge_index, edge_attr, x):
    """Host-side bucketing/sharding. Returns (in_maps_partial, meta)."""
    import ml_dtypes

    n_nodes = x.shape[0]
    n_edges = edge_index.shape[1]

    shift = BW.bit_length() - 1
    nb_total = -(-n_nodes // BW)  # buckets of BW nodes
    nb_total = -(-nb_total // N_CORES) * N_CORES  # round to multiple of n_cores
    NB = nb_total // N_CORES  # buckets per core
    n_pad = nb_total * BW

    row = np.asarray(edge_index[0], dtype=np.int64)
    bucket = row >> shift
    order = np.argsort(bucket, kind="stable")
    counts = np.bincount(bucket, minlength=nb_total)

    # Sort each core's buckets by edge count (descending) so that one
    # tile-count pattern T_LIST (the per-position max over cores) fits all
    # cores with minimal padding; outputs are un-permuted on the host.
    counts_pc = counts.reshape(N_CORES, NB)
    perm_pc = np.argsort(-counts_pc, kind="stable")  # [cores, NB] pos -> local bkt
    sorted_counts = np.take_along_axis(counts_pc, perm_pc, axis=1)
    T_arr = np.maximum(1, -(-sorted_counts.max(axis=0) // P))  # [NB]
    T_LIST = tuple(int(t) for t in T_arr)
    ST = int(T_arr.sum())
    # per-position edge offsets within a core's padded edge array
    pos_edge_off = np.zeros(NB, np.int64)
    pos_edge_off[1:] = np.cumsum(T_arr[:-1] * P)
    e_core_pad = int(ST * P)

    # rank of each global bucket within its core's order
    rank = np.empty(nb_total, np.int64)
    core_idx = np.repeat(np.arange(N_CORES), NB)
    rank[(np.arange(nb_total) // NB) * NB + perm_pc.ravel()] = np.tile(
        np.arange(NB), N_CORES
    )

    starts = np.zeros(nb_total, np.int64)
    starts[1:] = np.cumsum(counts)[:-1]
    bs = bucket[order]
    dest = (
        (bs // NB) * e_core_pad
        + pos_edge_off[rank[bs]]
        + (np.arange(n_edges, dtype=np.int64) - starts[bs])
    )
    perm = np.full(N_CORES * e_core_pad, -1, np.int64)
    perm[dest] = order
    valid = perm >= 0
    perm_c = np.where(valid, perm, 0)

    attr_pad = np.ascontiguousarray(np.asarray(edge_attr, dtype=np.float32)[perm_c])
    hi_u, lo_u = _bf16_split(attr_pad)
    attr_hl = np.empty((N_CORES * e_core_pad, 2 * D), np.uint16)
    attr_hl[:, :D] = hi_u
    attr_hl[:, D:] = lo_u
    attr_flat = attr_hl.reshape(N_CORES, e_core_pad * 2 * D).view(ml_dtypes.bfloat16)

    idxl = np.where(
        valid, (row[perm_c] & (BW - 1)).astype(np.float32), np.float32(-1.0)
    ).astype(ml_dtypes.bfloat16)  # ids 0..BW-1 and -1 are exact in bf16
    idxl = idxl.reshape(N_CORES, e_core_pad)

    x_pad = np.zeros((n_pad, D), np.float32)
    x_pad[:n_nodes] = np.asarray(x, dtype=np.float32)
    x_blocks = x_pad.reshape(nb_total, BW, D)

    nn_core = NB * BW
    per_core = []
    for k in range(N_CORES):
        sel = k * NB + perm_pc[k]
        xt_k = np.ascontiguousarray(
            x_blocks[sel].transpose(2, 0, 1).reshape(D, nn_core)
        )
        # idx: [P, ST] with position j's [P, T_j] block at column SOFF[j]
        idx_k = np.empty((P, ST), ml_dtypes.bfloat16)
        off = 0
        for j in range(NB):
            tj = int(T_arr[j])
            blk = idxl[k, pos_edge_off[j] : pos_edge_off[j] + tj * P]
            idx_k[:, off : off + tj] = blk.reshape(P, tj)
            off += tj
        per_core.append({
            "attr": attr_flat[k],
            "idx": np.ascontiguousarray(idx_k),
            "xt": xt_k,
        })
    meta = dict(
        T_LIST=T_LIST, NB=NB, n_pad=n_pad, nn_core=nn_core, perm_pc=perm_pc
    )
    return per_core, meta


def kernel(edge_index, edge_attr, x, W1, b1, W2, b2):
    global LAST_RESULTS
    _ensure_path()
    import ml_dtypes
    from concourse.bass_utils import run_bass_kernel_spmd

    edge_index = np.asarray(edge_index)
    edge_attr = np.asarray(edge_attr, dtype=np.float32)
    x = np.asarray(x, dtype=np.float32)
    W1 = np.asarray(W1, dtype=np.float32)
    b1 = np.asarray(b1, dtype=np.float32)
    W2 = np.asarray(W2, dtype=np.float32)
    b2 = np.asarray(b2, dtype=np.float32)

    n_nodes = x.shape[0]
    per_core, meta = _prepare(edge_index, edge_attr, x)
    T_LIST, NB, nn_core = meta["T_LIST"], meta["NB"], meta["nn_core"]

    iota = np.ascontiguousarray(
        np.broadcast_to(
            np.arange(BW, dtype=np.float32).astype(ml_dtypes.bfloat16), (P, BW)
        )
    )
    shared = {
        "w1x": np.ascontiguousarray(W1[:D]),
        "w1a": np.ascontiguousarray(W1[D:]),
        "w2": np.ascontiguousarray(W2),
        "b1": np.ascontiguousarray(b1.reshape(D, 1)),
        "b2": np.ascontiguousarray(b2.reshape(D, 1)),
        "iota": iota,
    }
    in_maps = [{**pc, **shared} for pc in per_core]

    nc = _build_program(T_LIST, NB)
    trace = bool(int(os.environ.get("KBENCH_TRACE", "0")))
    LAST_RESULTS = run_bass_kernel_spmd(
        nc, in_maps, list(range(N_CORES)), trace=trace
    )
    results = LAST_RESULTS.results

    perm_pc = meta["perm_pc"]
    agg = np.empty((meta["n_pad"] // BW, BW, D), np.float32)
    out = np.empty_like(agg)
    for k in range(N_CORES):
        sel = k * NB + perm_pc[k]
        agg[sel] = results[k]["aggt"].T.reshape(NB, BW, D)
        out[sel] = results[k]["outt"].T.reshape(NB, BW, D)
    agg = agg.reshape(-1, D)
    out = out.reshape(-1, D)

    combined = np.concatenate([x, agg[:n_nodes]], axis=1)
    return out[:n_nodes], combined


# revision 22
# speedup vs baseline: 2.2198x; 1.0123x over previous
"""GNN message-passing (NodeModel) Trainium2 kernel.

Strategy ("shard nodes, bucket edges" — a refinement of the edge-sharding hint
that removes the all-reduce entirely):
  * Host buckets edges by destination-node bucket of BW=64 nodes (a counting
    sort by `row >> 6`).  Buckets are distributed contiguously over the 8
    cores, so each core owns a contiguous 1/8 slice of the node space and
    ALL edges that point into it.  No cross-core reduction is needed.
  * On device, each bucket's segment-sum runs on the tensor engine: for each
    128-edge tile, a one-hot "placement" matrix P[e, n] = (local_id[e] == n)
    is built with a DVE is_equal against an iota row, and
    aggT += attr_tile.T @ P accumulates in PSUM.  Padding edges carry local
    id -1 and contribute nothing.  BW=64 keeps the placement build (the DVE
    cost) at half of what 128-wide buckets would need.
  * Precision: edge attrs are split on the host into bf16 hi + lo parts
    (attr ≈ hi + lo to ~2^-16 relative).  The stationary operand packs
    [hi | lo] as 128 columns, so ONE bf16 matmul per edge tile produces
    both partial aggregates (PSUM rows 0-63 = hi, 64-127 = lo); an ACT copy
    plus DVE add folds them to fp32.  This halves tensor-engine time vs
    fp32 matmuls (which lower to 2 HW passes) at ~1e-5 relative accuracy.
  * The 2-layer MLP runs on the same core over its node slice, entirely in
    feature-major (transposed) layout:  hT = silu(W1x.T@xT + W1a.T@aggT + b1),
    outT = W2.T @ hT + b2  (fp32), batched 512 nodes per matmul.
  * Device outputs are feature-major [64, nodes/core]; the host transposes
    and assembles the full (out, combined) pair.
"""

import functools
import os
import sys

import numpy as np


def _ensure_path():
    try:
        import concourse  # noqa: F401
    except ImportError:
        for p in ("/opt/trn_rl_repo", "/root/.axon_site/_ro/trn_rl_repo"):
            if os.path.isdir(p):
                sys.path.insert(0, p)
                break


P = 128  # edge-tile size (contraction dim)
D = 64  # feature dim
BW = 128  # nodes per bucket (one-hot width)
N_CORES = 8

# Stash of the last BassKernelResults (for test harness introspection).
LAST_RESULTS = None


def _bf16_split(a):
    """Round-to-nearest-even split of fp32 `a` into bf16 hi/lo bit patterns."""
    u = a.view(np.uint32)
    hi_u = ((u + (((u >> 16) & 1) + 0x7FFF)) >> 16).astype(np.uint16)
    hi_f = (hi_u.astype(np.uint32) << 16).view(np.float32)
    r = np.asarray(a - hi_f, dtype=np.float32)
    ur = r.view(np.uint32)
    lo_u = ((ur + (((ur >> 16) & 1) + 0x7FFF)) >> 16).astype(np.uint16)
    return hi_u, lo_u


@functools.lru_cache(maxsize=None)
def _build_program(T_LIST: tuple, NB: int):
    """Build the Bass program.

    T_LIST = per-position edge-tile counts (buckets are sorted by edge count
             on the host so one descending tile-count pattern fits all cores)
    NB     = node buckets (of BW nodes) per core
    """
    _ensure_path()
    import concourse.tile as tile
    from concourse import bacc, mybir
    from contextlib import ExitStack

    f32 = mybir.dt.float32
    bf16 = mybir.dt.bfloat16
    NN = NB * BW  # nodes per core
    ST = sum(T_LIST)
    SOFF = [0] * NB  # per-position tile offsets
    for j in range(1, NB):
        SOFF[j] = SOFF[j - 1] + T_LIST[j - 1]
    Tmax = max(T_LIST)

    nc = bacc.Bacc("TRN2", target_bir_lowering=False, debug=False)

    attr_d = nc.declare_dram_parameter("attr", [P * ST * P], bf16, isOutput=False)
    idx_d = nc.declare_dram_parameter("idx", [P, ST], bf16, isOutput=False)
    xt_d = nc.declare_dram_parameter("xt", [D, NN], f32, isOutput=False)
    w1x_d = nc.declare_dram_parameter("w1x", [D, D], f32, isOutput=False)
    w1a_d = nc.declare_dram_parameter("w1a", [D, D], f32, isOutput=False)
    w2_d = nc.declare_dram_parameter("w2", [D, D], f32, isOutput=False)
    b1_d = nc.declare_dram_parameter("b1", [D, 1], f32, isOutput=False)
    b2_d = nc.declare_dram_parameter("b2", [D, 1], f32, isOutput=False)
    iota_d = nc.declare_dram_parameter("iota", [P, BW], bf16, isOutput=False)
    aggt_o = nc.declare_dram_parameter("aggt", [D, NN], f32, isOutput=True)
    outt_o = nc.declare_dram_parameter("outt", [D, NN], f32, isOutput=True)

    with tile.TileContext(nc) as tc, ExitStack() as ctx:
        consts = ctx.enter_context(tc.tile_pool(name="consts", bufs=1))
        attr_pool = ctx.enter_context(tc.tile_pool(name="attr", bufs=5))
        plc_pool = ctx.enter_context(tc.tile_pool(name="plc", bufs=5))
        rep_pool = ctx.enter_context(tc.tile_pool(name="rep", bufs=5))
        sbout_pool = ctx.enter_context(tc.tile_pool(name="sbout", bufs=4))
        ps_agg = ctx.enter_context(tc.tile_pool(name="ps_agg", bufs=3, space="PSUM"))
        ps_mlp = ctx.enter_context(tc.tile_pool(name="ps_mlp", bufs=2, space="PSUM"))

        iota_sb = consts.tile([P, BW], bf16)
        nc.scalar.dma_start(iota_sb[:], iota_d[:, :])
        b1_sb = consts.tile([D, 1], f32)
        nc.scalar.dma_start(b1_sb[:], b1_d[:, :])
        b2_sb = consts.tile([D, 1], f32)
        nc.scalar.dma_start(b2_sb[:], b2_d[:, :])
        idx_sb = consts.tile([P, ST], bf16)
        nc.scalar.dma_start(idx_sb[:], idx_d[:, :])
        w1x_sb = consts.tile([D, D], f32)
        nc.scalar.dma_start(w1x_sb[:], w1x_d[:, :])
        w1a_sb = consts.tile([D, D], f32)
        nc.scalar.dma_start(w1a_sb[:], w1a_d[:, :])
        w2_sb = consts.tile([D, D], f32)
        nc.scalar.dma_start(w2_sb[:], w2_d[:, :])
        xt_sb = consts.tile([D, NN], f32)
        nc.scalar.dma_start(xt_sb[:], xt_d[:, :])

        # MLP is batched over groups of buckets (up to 512 nodes per matmul)
        G = 512 // BW
        groups = [list(range(g0, min(g0 + G, NB))) for g0 in range(0, NB, G)]
        for grp in groups:
            W = len(grp) * BW
            agg4 = sbout_pool.tile([D, G * BW], f32, tag="agg")
            for q, b in enumerate(grp):
                Tj = T_LIST[b]
                o0 = P * P * SOFF[b]
                attr_sb = attr_pool.tile([P, Tmax * P], bf16)
                nc.sync.dma_start(
                    attr_sb[:, : Tj * P],
                    attr_d[o0 : o0 + P * Tj * P].rearrange("(p c) -> p c", p=P),
                )

                # one batched op builds all Tj placement one-hots.  A
                # stride-0 (broadcast) operand forces the DVE TensorTensor
                # into 1x mode, so for 2/3 of buckets the ACT engine first
                # materializes idx_rep; the DVE compare then has unit-stride
                # bf16 operands and runs in 2x_1P mode.
                plc = plc_pool.tile([P, Tmax, BW], bf16)
                idx_b = (
                    idx_sb[:, SOFF[b] : SOFF[b] + Tj]
                    .unsqueeze(2)
                    .to_broadcast([P, Tj, BW])
                )
                iota_b = iota_sb[:].unsqueeze(1).to_broadcast([P, Tj, BW])
                if b % 3 == 0:  # mode A: direct 1x compare on DVE
                    nc.vector.tensor_tensor(
                        out=plc[:, :Tj, :], in0=idx_b, in1=iota_b,
                        op=mybir.AluOpType.is_equal,
                    )
                else:  # mode B: ACT materializes the broadcast, DVE compares 2x
                    idx_rep = rep_pool.tile([P, Tmax, BW], bf16)
                    nc.scalar.activation(
                        out=idx_rep[:, :Tj, :], in_=idx_b,
                        func=mybir.ActivationFunctionType.Copy,
                    )
                    nc.vector.tensor_tensor(
                        out=plc[:, :Tj, :], in0=idx_rep[:, :Tj, :], in1=iota_b,
                        op=mybir.AluOpType.is_equal,
                    )

                agg_ps = ps_agg.tile([P, BW], f32)
                for t in range(Tj):
                    nc.tensor.matmul(
                        out=agg_ps[:],
                        lhsT=attr_sb[:, t * P : (t + 1) * P],
                        rhs=plc[:, t, :],
                        start=(t == 0),
                        stop=(t == Tj - 1),
                    )

                # fold hi (rows 0-63) + lo (rows 64-127) partial sums to fp32
                # (only one operand may be in PSUM: stage hi through ACT first)
                agg_hi = sbout_pool.tile([D, BW], f32, tag="agghi")
                nc.scalar.activation(
                    out=agg_hi[:], in_=agg_ps[0:D, :],
                    func=mybir.ActivationFunctionType.Copy,
                )
                nc.vector.tensor_add(
                    agg4[:, q * BW : (q + 1) * BW], agg_hi[:], agg_ps[D : 2 * D, :]
                )

            c0 = grp[0] * BW
            nc.scalar.dma_start(aggt_o[:, c0 : c0 + W], agg4[:, :W])

            h_ps = ps_mlp.tile([D, G * BW], f32, tag="h")
            nc.tensor.matmul(
                out=h_ps[:, :W], lhsT=w1x_sb[:], rhs=xt_sb[:, c0 : c0 + W],
                start=True, stop=False,
            )
            nc.tensor.matmul(
                out=h_ps[:, :W], lhsT=w1a_sb[:], rhs=agg4[:, :W],
                start=False, stop=True,
            )
            h_sb = sbout_pool.tile([D, G * BW], f32, tag="hsb")
            nc.scalar.activation(
                out=h_sb[:, :W], in_=h_ps[:, :W],
                func=mybir.ActivationFunctionType.Silu,
                bias=b1_sb[:],
            )
            o_ps = ps_mlp.tile([D, G * BW], f32, tag="o")
            nc.tensor.matmul(
                out=o_ps[:, :W], lhsT=w2_sb[:], rhs=h_sb[:, :W],
                start=True, stop=True,
            )
            o_sb = sbout_pool.tile([D, G * BW], f32, tag="osb")
            nc.scalar.activation(
                out=o_sb[:, :W], in_=o_ps[:, :W],
                func=mybir.ActivationFunctionType.Identity,
                bias=b2_sb[:],
            )
            nc.scalar.dma_start(outt_o[:, c0 : c0 + W], o_sb[:, :W])

    nc.compile()
    return nc


def _prepare(edge_index, edge_attr, x):
    """Host-side bucketing/sharding. Returns (in_maps_partial, meta)."""
    import ml_dtypes

    n_nodes = x.shape[0]
    n_edges = edge_index.shape[1]

    shift = BW.bit_length() - 1
    nb_total = -(-n_nodes // BW)  # buckets of BW nodes
    nb_total = -(-nb_total // N_CORES) * N_CORES  # round to multiple of n_cores
    NB = nb_total // N_CORES  # buckets per core
    n_pad = nb_total * BW

    row = np.asarray(edge_index[0], dtype=np.int64)
    bucket = row >> shift
    order = np.argsort(bucket, kind="stable")
    counts = np.bincount(bucket, minlength=nb_total)

    # Sort each core's buckets by edge count (descending) so that one
    # tile-count pattern T_LIST (the per-position max over cores) fits all
    # cores with minimal padding; outputs are un-permuted on the host.
    counts_pc = counts.reshape(N_CORES, NB)
    perm_pc = np.argsort(-counts_pc, kind="stable")  # [cores, NB] pos -> local bkt
    sorted_counts = np.take_along_axis(counts_pc, perm_pc, axis=1)
    T_arr = np.maximum(1, -(-sorted_counts.max(axis=0) // P))  # [NB]
    T_LIST = tuple(int(t) for t in T_arr)
    ST = int(T_arr.sum())
    # per-position edge offsets within a core's padded edge array
    pos_edge_off = np.zeros(NB, np.int64)
    pos_edge_off[1:] = np.cumsum(T_arr[:-1] * P)
    e_core_pad = int(ST * P)

    # rank of each global bucket within its core's order
    rank = np.empty(nb_total, np.int64)
    core_idx = np.repeat(np.arange(N_CORES), NB)
    rank[(np.arange(nb_total) // NB) * NB + perm_pc.ravel()] = np.tile(
        np.arange(NB), N_CORES
    )

    starts = np.zeros(nb_total, np.int64)
    starts[1:] = np.cumsum(counts)[:-1]
    bs = bucket[order]
    dest = (
        (bs // NB) * e_core_pad
        + pos_edge_off[rank[bs]]
        + (np.arange(n_edges, dtype=np.int64) - starts[bs])
    )
    perm = np.full(N_CORES * e_core_pad, -1, np.int64)
    perm[dest] = order
    valid = perm >= 0
    perm_c = np.where(valid, perm, 0)

    attr_pad = np.ascontiguousarray(np.asarray(edge_attr, dtype=np.float32)[perm_c])
    hi_u, lo_u = _bf16_split(attr_pad)
    attr_hl = np.empty((N_CORES * e_core_pad, 2 * D), np.uint16)
    attr_hl[:, :D] = hi_u
    attr_hl[:, D:] = lo_u
    attr_flat = attr_hl.reshape(N_CORES, e_core_pad * 2 * D).view(ml_dtypes.bfloat16)

    idxl = np.where(
        valid, (row[perm_c] & (BW - 1)).astype(np.float32), np.float32(-1.0)
    ).astype(ml_dtypes.bfloat16)  # ids 0..BW-1 and -1 are exact in bf16
    idxl = idxl.reshape(N_CORES, e_core_pad)

    x_pad = np.zeros((n_pad, D), np.float32)
    x_pad[:n_nodes] = np.asarray(x, dtype=np.float32)
    x_blocks = x_pad.reshape(nb_total, BW, D)

    nn_core = NB * BW
    per_core = []
    for k in range(N_CORES):
        sel = k * NB + perm_pc[k]
        xt_k = np.ascontiguousarray(
            x_blocks[sel].transpose(2, 0, 1).reshape(D, nn_core)
        )
        # idx: [P, ST] with position j's [P, T_j] block at column SOFF[j]
        idx_k = np.empty((P, ST), ml_dtypes.bfloat16)
        off = 0
        for j in range(NB):
            tj = int(T_arr[j])
            blk = idxl[k, pos_edge_off[j] : pos_edge_off[j] + tj * P]
            idx_k[:, off : off + tj] = blk.reshape(P, tj)
            off += tj
        per_core.append({
            "attr": attr_flat[k],
            "idx": np.ascontiguousarray(idx_k),
            "xt": xt_k,
        })
    meta = dict(
        T_LIST=T_LIST, NB=NB, n_pad=n_pad, nn_core=nn_core, perm_pc=perm_pc
    )
    return per_core, meta


def kernel(edge_index, edge_attr, x, W1, b1, W2, b2):
    global LAST_RESULTS
    _ensure_path()
    import ml_dtypes
    from concourse.bass_utils import run_bass_kernel_spmd

    edge_index = np.asarray(edge_index)
    edge_attr = np.asarray(edge_attr, dtype=np.float32)
    x = np.asarray(x, dtype=np.float32)
    W1 = np.asarray(W1, dtype=np.float32)
    b1 = np.asarray(b1, dtype=np.float32)
    W2 = np.asarray(W2, dtype=np.float32)
    b2 = np.asarray(b2, dtype=np.float32)

    n_nodes = x.shape[0]
    per_core, meta = _prepare(edge_index, edge_attr, x)
    T_LIST, NB, nn_core = meta["T_LIST"], meta["NB"], meta["nn_core"]

    iota = np.ascontiguousarray(
        np.broadcast_to(
            np.arange(BW, dtype=np.float32).astype(ml_dtypes.bfloat16), (P, BW)
        )
    )
    shared = {
        "w1x": np.ascontiguousarray(W1[:D]),
        "w1a": np.ascontiguousarray(W1[D:]),
        "w2": np.ascontiguousarray(W2),
        "b1": np.ascontiguousarray(b1.reshape(D, 1)),
        "b2": np.ascontiguousarray(b2.reshape(D, 1)),
        "iota": iota,
    }
    in_maps = [{**pc, **shared} for pc in per_core]

    nc = _build_program(T_LIST, NB)
    trace = bool(int(os.environ.get("KBENCH_TRACE", "0")))
    LAST_RESULTS = run_bass_kernel_spmd(
        nc, in_maps, list(range(N_CORES)), trace=trace
    )
    results = LAST_RESULTS.results

    perm_pc = meta["perm_pc"]
    agg = np.empty((meta["n_pad"] // BW, BW, D), np.float32)
    out = np.empty_like(agg)
    for k in range(N_CORES):
        sel = k * NB + perm_pc[k]
        agg[sel] = results[k]["aggt"].T.reshape(NB, BW, D)
        out[sel] = results[k]["outt"].T.reshape(NB, BW, D)
    agg = agg.reshape(-1, D)
    out = out.reshape(-1, D)

    combined = np.concatenate([x, agg[:n_nodes]], axis=1)
    return out[:n_nodes], combined


# revision 23
# speedup vs baseline: 2.2602x; 1.0182x over previous
"""GNN message-passing (NodeModel) Trainium2 kernel.

Strategy ("shard nodes, bucket edges" — a refinement of the edge-sharding hint
that removes the all-reduce entirely):
  * Host buckets edges by destination-node bucket of BW=64 nodes (a counting
    sort by `row >> 6`).  Buckets are distributed contiguously over the 8
    cores, so each core owns a contiguous 1/8 slice of the node space and
    ALL edges that point into it.  No cross-core reduction is needed.
  * On device, each bucket's segment-sum runs on the tensor engine: for each
    128-edge tile, a one-hot "placement" matrix P[e, n] = (local_id[e] == n)
    is built with a DVE is_equal against an iota row, and
    aggT += attr_tile.T @ P accumulates in PSUM.  Padding edges carry local
    id -1 and contribute nothing.  BW=64 keeps the placement build (the DVE
    cost) at half of what 128-wide buckets would need.
  * Precision: edge attrs are split on the host into bf16 hi + lo parts
    (attr ≈ hi + lo to ~2^-16 relative).  The stationary operand packs
    [hi | lo] as 128 columns, so ONE bf16 matmul per edge tile produces
    both partial aggregates (PSUM rows 0-63 = hi, 64-127 = lo); an ACT copy
    plus DVE add folds them to fp32.  This halves tensor-engine time vs
    fp32 matmuls (which lower to 2 HW passes) at ~1e-5 relative accuracy.
  * The 2-layer MLP runs on the same core over its node slice, entirely in
    feature-major (transposed) layout:  hT = silu(W1x.T@xT + W1a.T@aggT + b1),
    outT = W2.T @ hT + b2  (fp32), batched 512 nodes per matmul.
  * Device outputs are feature-major [64, nodes/core]; the host transposes
    and assembles the full (out, combined) pair.
"""

import functools
import os
import sys

import numpy as np


def _ensure_path():
    try:
        import concourse  # noqa: F401
    except ImportError:
        for p in ("/opt/trn_rl_repo", "/root/.axon_site/_ro/trn_rl_repo"):
            if os.path.isdir(p):
                sys.path.insert(0, p)
                break


P = 128  # edge-tile size (contraction dim)
D = 64  # feature dim
BW = 128  # nodes per bucket (one-hot width)
N_CORES = 8

# Stash of the last BassKernelResults (for test harness introspection).
LAST_RESULTS = None


def _bf16_split(a):
    """Round-to-nearest-even split of fp32 `a` into bf16 hi/lo bit patterns."""
    u = a.view(np.uint32)
    hi_u = ((u + (((u >> 16) & 1) + 0x7FFF)) >> 16).astype(np.uint16)
    hi_f = (hi_u.astype(np.uint32) << 16).view(np.float32)
    r = np.asarray(a - hi_f, dtype=np.float32)
    ur = r.view(np.uint32)
    lo_u = ((ur + (((ur >> 16) & 1) + 0x7FFF)) >> 16).astype(np.uint16)
    return hi_u, lo_u


@functools.lru_cache(maxsize=None)
def _build_program(T_LIST: tuple, NB: int):
    """Build the Bass program.

    T_LIST = per-position edge-tile counts (buckets are sorted by edge count
             on the host so one descending tile-count pattern fits all cores)
    NB     = node buckets (of BW nodes) per core
    """
    _ensure_path()
    import concourse.tile as tile
    from concourse import bacc, mybir
    from contextlib import ExitStack

    f32 = mybir.dt.float32
    bf16 = mybir.dt.bfloat16
    NN = NB * BW  # nodes per core
    ST = sum(T_LIST)
    SOFF = [0] * NB  # per-position tile offsets
    for j in range(1, NB):
        SOFF[j] = SOFF[j - 1] + T_LIST[j - 1]
    Tmax = max(T_LIST)

    nc = bacc.Bacc("TRN2", target_bir_lowering=False, debug=False)

    attr_d = nc.declare_dram_parameter("attr", [P * ST * P], bf16, isOutput=False)
    idx_d = nc.declare_dram_parameter("idx", [P, ST], bf16, isOutput=False)
    xt_d = nc.declare_dram_parameter("xt", [D, NN], f32, isOutput=False)
    w1x_d = nc.declare_dram_parameter("w1x", [D, D], f32, isOutput=False)
    w1a_d = nc.declare_dram_parameter("w1a", [D, D], f32, isOutput=False)
    w2_d = nc.declare_dram_parameter("w2", [D, D], f32, isOutput=False)
    b1_d = nc.declare_dram_parameter("b1", [D, 1], f32, isOutput=False)
    b2_d = nc.declare_dram_parameter("b2", [D, 1], f32, isOutput=False)
    iota_d = nc.declare_dram_parameter("iota", [P, BW], bf16, isOutput=False)
    aggt_o = nc.declare_dram_parameter("aggt", [D, NN], f32, isOutput=True)
    outt_o = nc.declare_dram_parameter("outt", [D, NN], f32, isOutput=True)

    with tile.TileContext(nc) as tc, ExitStack() as ctx:
        consts = ctx.enter_context(tc.tile_pool(name="consts", bufs=1))
        attr_pool = ctx.enter_context(tc.tile_pool(name="attr", bufs=6))
        plc_pool = ctx.enter_context(tc.tile_pool(name="plc", bufs=5))
        rep_pool = ctx.enter_context(tc.tile_pool(name="rep", bufs=5))
        sbout_pool = ctx.enter_context(tc.tile_pool(name="sbout", bufs=4))
        ps_agg = ctx.enter_context(tc.tile_pool(name="ps_agg", bufs=3, space="PSUM"))
        ps_mlp = ctx.enter_context(tc.tile_pool(name="ps_mlp", bufs=2, space="PSUM"))

        iota_sb = consts.tile([P, BW], bf16)
        nc.scalar.dma_start(iota_sb[:], iota_d[:, :])
        b1_sb = consts.tile([D, 1], f32)
        nc.scalar.dma_start(b1_sb[:], b1_d[:, :])
        b2_sb = consts.tile([D, 1], f32)
        nc.scalar.dma_start(b2_sb[:], b2_d[:, :])
        idx_sb = consts.tile([P, ST], bf16)
        nc.scalar.dma_start(idx_sb[:], idx_d[:, :])
        w1x_sb = consts.tile([D, D], f32)
        nc.scalar.dma_start(w1x_sb[:], w1x_d[:, :])
        w1a_sb = consts.tile([D, D], f32)
        nc.scalar.dma_start(w1a_sb[:], w1a_d[:, :])
        w2_sb = consts.tile([D, D], f32)
        nc.scalar.dma_start(w2_sb[:], w2_d[:, :])
        xt_sb = consts.tile([D, NN], f32)
        nc.scalar.dma_start(xt_sb[:], xt_d[:, :])

        # MLP is batched over groups of buckets (up to 512 nodes per matmul)
        G = 512 // BW
        groups = [list(range(g0, min(g0 + G, NB))) for g0 in range(0, NB, G)]
        for grp in groups:
            W = len(grp) * BW
            agg4 = sbout_pool.tile([D, G * BW], f32, tag="agg")
            for q, b in enumerate(grp):
                Tj = T_LIST[b]
                o0 = P * P * SOFF[b]
                attr_sb = attr_pool.tile([P, Tmax * P], bf16)
                nc.sync.dma_start(
                    attr_sb[:, : Tj * P],
                    attr_d[o0 : o0 + P * Tj * P].rearrange("(p c) -> p c", p=P),
                )

                # one batched op builds all Tj placement one-hots.  A
                # stride-0 (broadcast) operand forces the DVE TensorTensor
                # into 1x mode, so for 2/3 of buckets the ACT engine first
                # materializes idx_rep; the DVE compare then has unit-stride
                # bf16 operands and runs in 2x_1P mode.
                plc = plc_pool.tile([P, Tmax, BW], bf16)
                idx_b = (
                    idx_sb[:, SOFF[b] : SOFF[b] + Tj]
                    .unsqueeze(2)
                    .to_broadcast([P, Tj, BW])
                )
                iota_b = iota_sb[:].unsqueeze(1).to_broadcast([P, Tj, BW])
                if b % 3 == 0:  # mode A: direct 1x compare on DVE
                    nc.vector.tensor_tensor(
                        out=plc[:, :Tj, :], in0=idx_b, in1=iota_b,
                        op=mybir.AluOpType.is_equal,
                    )
                else:  # mode B: ACT materializes the broadcast, DVE compares 2x
                    idx_rep = rep_pool.tile([P, Tmax, BW], bf16)
                    nc.scalar.activation(
                        out=idx_rep[:, :Tj, :], in_=idx_b,
                        func=mybir.ActivationFunctionType.Copy,
                    )
                    nc.vector.tensor_tensor(
                        out=plc[:, :Tj, :], in0=idx_rep[:, :Tj, :], in1=iota_b,
                        op=mybir.AluOpType.is_equal,
                    )

                agg_ps = ps_agg.tile([P, BW], f32)
                for t in range(Tj):
                    nc.tensor.matmul(
                        out=agg_ps[:],
                        lhsT=attr_sb[:, t * P : (t + 1) * P],
                        rhs=plc[:, t, :],
                        start=(t == 0),
                        stop=(t == Tj - 1),
                    )

                # fold hi (rows 0-63) + lo (rows 64-127) partial sums to fp32
                # (only one operand may be in PSUM: stage hi through ACT first)
                agg_hi = sbout_pool.tile([D, BW], f32, tag="agghi")
                nc.scalar.activation(
                    out=agg_hi[:], in_=agg_ps[0:D, :],
                    func=mybir.ActivationFunctionType.Copy,
                )
                nc.vector.tensor_add(
                    agg4[:, q * BW : (q + 1) * BW], agg_hi[:], agg_ps[D : 2 * D, :]
                )

            c0 = grp[0] * BW
            nc.scalar.dma_start(aggt_o[:, c0 : c0 + W], agg4[:, :W])

            h_ps = ps_mlp.tile([D, G * BW], f32, tag="h")
            nc.tensor.matmul(
                out=h_ps[:, :W], lhsT=w1x_sb[:], rhs=xt_sb[:, c0 : c0 + W],
                start=True, stop=False,
            )
            nc.tensor.matmul(
                out=h_ps[:, :W], lhsT=w1a_sb[:], rhs=agg4[:, :W],
                start=False, stop=True,
            )
            h_sb = sbout_pool.tile([D, G * BW], f32, tag="hsb")
            nc.scalar.activation(
                out=h_sb[:, :W], in_=h_ps[:, :W],
                func=mybir.ActivationFunctionType.Silu,
                bias=b1_sb[:],
            )
            o_ps = ps_mlp.tile([D, G * BW], f32, tag="o")
            nc.tensor.matmul(
                out=o_ps[:, :W], lhsT=w2_sb[:], rhs=h_sb[:, :W],
                start=True, stop=True,
            )
            o_sb = sbout_pool.tile([D, G * BW], f32, tag="osb")
            nc.scalar.activation(
                out=o_sb[:, :W], in_=o_ps[:, :W],
                func=mybir.ActivationFunctionType.Identity,
                bias=b2_sb[:],
            )
            nc.scalar.dma_start(outt_o[:, c0 : c0 + W], o_sb[:, :W])

    nc.compile()
    return nc


def _prepare(edge_index, edge_attr, x):
    """Host-side bucketing/sharding. Returns (in_maps_partial, meta)."""
    import ml_dtypes

    n_nodes = x.shape[0]
    n_edges = edge_index.shape[1]

    shift = BW.bit_length() - 1
    nb_total = -(-n_nodes // BW)  # buckets of BW nodes
    nb_total = -(-nb_total // N_CORES) * N_CORES  # round to multiple of n_cores
    NB = nb_total // N_CORES  # buckets per core
    n_pad = nb_total * BW

    row = np.asarray(edge_index[0], dtype=np.int64)
    bucket = row >> shift
    order = np.argsort(bucket, kind="stable")
    counts = np.bincount(bucket, minlength=nb_total)

    # Sort each core's buckets by edge count (descending) so that one
    # tile-count pattern T_LIST (the per-position max over cores) fits all
    # cores with minimal padding; outputs are un-permuted on the host.
    counts_pc = counts.reshape(N_CORES, NB)
    perm_pc = np.argsort(-counts_pc, kind="stable")  # [cores, NB] pos -> local bkt
    sorted_counts = np.take_along_axis(counts_pc, perm_pc, axis=1)
    T_arr = np.maximum(1, -(-sorted_counts.max(axis=0) // P))  # [NB]
    T_LIST = tuple(int(t) for t in T_arr)
    ST = int(T_arr.sum())
    # per-position edge offsets within a core's padded edge array
    pos_edge_off = np.zeros(NB, np.int64)
    pos_edge_off[1:] = np.cumsum(T_arr[:-1] * P)
    e_core_pad = int(ST * P)

    # rank of each global bucket within its core's order
    rank = np.empty(nb_total, np.int64)
    core_idx = np.repeat(np.arange(N_CORES), NB)
    rank[(np.arange(nb_total) // NB) * NB + perm_pc.ravel()] = np.tile(
        np.arange(NB), N_CORES
    )

    starts = np.zeros(nb_total, np.int64)
    starts[1:] = np.cumsum(counts)[:-1]
    bs = bucket[order]
    dest = (
        (bs // NB) * e_core_pad
        + pos_edge_off[rank[bs]]
        + (np.arange(n_edges, dtype=np.int64) - starts[bs])
    )
    perm = np.full(N_CORES * e_core_pad, -1, np.int64)
    perm[dest] = order
    valid = perm >= 0
    perm_c = np.where(valid, perm, 0)

    attr_pad = np.ascontiguousarray(np.asarray(edge_attr, dtype=np.float32)[perm_c])
    hi_u, lo_u = _bf16_split(attr_pad)
    attr_hl = np.empty((N_CORES * e_core_pad, 2 * D), np.uint16)
    attr_hl[:, :D] = hi_u
    attr_hl[:, D:] = lo_u
    attr_flat = attr_hl.reshape(N_CORES, e_core_pad * 2 * D).view(ml_dtypes.bfloat16)

    idxl = np.where(
        valid, (row[perm_c] & (BW - 1)).astype(np.float32), np.float32(-1.0)
    ).astype(ml_dtypes.bfloat16)  # ids 0..BW-1 and -1 are exact in bf16
    idxl = idxl.reshape(N_CORES, e_core_pad)

    x_pad = np.zeros((n_pad, D), np.float32)
    x_pad[:n_nodes] = np.asarray(x, dtype=np.float32)
    x_blocks = x_pad.reshape(nb_total, BW, D)

    nn_core = NB * BW
    per_core = []
    for k in range(N_CORES):
        sel = k * NB + perm_pc[k]
        xt_k = np.ascontiguousarray(
            x_blocks[sel].transpose(2, 0, 1).reshape(D, nn_core)
        )
        # idx: [P, ST] with position j's [P, T_j] block at column SOFF[j]
        idx_k = np.empty((P, ST), ml_dtypes.bfloat16)
        off = 0
        for j in range(NB):
            tj = int(T_arr[j])
            blk = idxl[k, pos_edge_off[j] : pos_edge_off[j] + tj * P]
            idx_k[:, off : off + tj] = blk.reshape(P, tj)
            off += tj
        per_core.append({
            "attr": attr_flat[k],
            "idx": np.ascontiguousarray(idx_k),
            "xt": xt_k,
        })
    meta = dict(
        T_LIST=T_LIST, NB=NB, n_pad=n_pad, nn_core=nn_core, perm_pc=perm_pc
    )
    return per_core, meta


def kernel(edge_index, edge_attr, x, W1, b1, W2, b2):
    global LAST_RESULTS
    _ensure_path()
    import ml_dtypes
    from concourse.bass_utils import run_bass_kernel_spmd

    edge_index = np.asarray(edge_index)
    edge_attr = np.asarray(edge_attr, dtype=np.float32)
    x = np.asarray(x, dtype=np.float32)
    W1 = np.asarray(W1, dtype=np.float32)
    b1 = np.asarray(b1, dtype=np.float32)
    W2 = np.asarray(W2, dtype=np.float32)
    b2 = np.asarray(b2, dtype=np.float32)

    n_nodes = x.shape[0]
    per_core, meta = _prepare(edge_index, edge_attr, x)
    T_LIST, NB, nn_core = meta["T_LIST"], meta["NB"], meta["nn_core"]

    iota = np.ascontiguousarray(
        np.broadcast_to(
            np.arange(BW, dtype=np.float32).astype(ml_dtypes.bfloat16), (P, BW)
        )
    )
    shared = {
        "w1x": np.ascontiguousarray(W1[:D]),
        "w1a": np.ascontiguousarray(W1[D:]),
        "w2": np.ascontiguousarray(W2),
        "b1": np.ascontiguousarray(b1.reshape(D, 1)),
        "b2": np.ascontiguousarray(b2.reshape(D, 1)),
        "iota": iota,
    }
    in_maps = [{**pc, **shared} for pc in per_core]

    nc = _build_program(T_LIST, NB)
    trace = bool(int(os.environ.get("KBENCH_TRACE", "0")))
    LAST_RESULTS = run_bass_kernel_spmd(
        nc, in_maps, list(range(N_CORES)), trace=trace
    )
    results = LAST_RESULTS.results

    perm_pc = meta["perm_pc"]
    agg = np.empty((meta["n_pad"] // BW, BW, D), np.float32)
    out = np.empty_like(agg)
    for k in range(N_CORES):
        sel = k * NB + perm_pc[k]
        agg[sel] = results[k]["aggt"].T.reshape(NB, BW, D)
        out[sel] = results[k]["outt"].T.reshape(NB, BW, D)
    agg = agg.reshape(-1, D)
    out = out.reshape(-1, D)

    combined = np.concatenate([x, agg[:n_nodes]], axis=1)
    return out[:n_nodes], combined
